# revision 15
# baseline (speedup 1.0000x reference)
"""Trainium2 Bass kernel for the MoE feed-forward block (nn_MoEFF).

Strategy: pure data-parallel over tokens. The 4096 tokens are split into
8 slices of 512; each NeuronCore runs the *entire* network on its slice
(router + all 8 experts dense-masked + shared expert). With E=8/K=4 every
expert serves ~half the tokens anyway, so dense-masked expert compute
costs only 2x the ideal sparse flops and avoids all collectives.

Precision: the front block (lin0 + swiglu1), router, all routed experts
and the shared expert run in fp8 e4m3 with DoubleRow matmuls (2 fp8
weights per PE cell, 256-deep contraction per instruction -> ~1.76x
bf16 matmul throughput measured). Activations and weights carry static
power-of-2 scales chosen so sigma*scale ~ 8-20 (TRN e4m3 max normal is
240; every descale constant folds into an existing activation/vector
op). The back block (lin1 + swiglu2) stays bf16: fp8 noise there lands
directly on the output with no averaging downstream, and it is only
3/36 of the matmul work. Measured end-to-end rel err 7.1e-3.

Layout: activations are kept transposed in SBUF ([feature-on-partition,
token-on-free]) so weight matrices in natural [in, out] layout are the
stationary matmul operand. Weights are host-packed p-major so every
DMA row is contiguous; lin0 additionally ships n-major so its first
column chunk (and the first matmul) starts ~4us earlier. Each expert's
down-projection is emitted one expert late so its PSUM->fp8 vector
chain hides under the next expert's up-projection matmuls. All PSUM
accumulation is fp32.
"""

from contextlib import ExitStack

import ml_dtypes
import numpy as np

B, S, D = 2, 2048, 1024
E, TOPK, H = 8, 4, 1024
SH = 2 * H
NCORES = 8
T = B * S                 # 4096 tokens
TPC = T // NCORES         # 512 tokens per core
KT = D // 128             # 8 contraction tiles
KT2 = KT // 2             # 4 DoubleRow contraction steps
MT_H = H // 128           # 8
MT_SH = SH // 128         # 16
NT = TPC // 128           # 4 token sub-tiles (router)

bf16 = ml_dtypes.bfloat16
f8 = ml_dtypes.float8_e4m3  # TRN FP8_EXP4-compatible (max normal 240)

# static power-of-2 scales (data ~N(0, sigma) with known sigmas)
SW = 512.0                # fp8 weights (sigma 0.02 -> 10)
SX = 16.0                 # x (sigma 1.0)
SH0 = 32.0                # h0 (sigma 0.64)
SHH = 64.0                # h, the MoE input (sigma 0.096, absmax 2.24)
SV = 4096.0               # expert/shared v = silu(g)*u (sigma 0.002)

# fp8 unit layout in wall8 (each unit is one [1024, 1024] matrix,
# host-packed to [128, KT*1024] p-major rows)
U_S1A = 0
U_S1B = 1
U_EXP = 2                 # 2 + 3*e + {0: w1, 1: w3, 2: w2}
U_SH1 = 26                # 26, 27: sh_w1 cols [0:1024], [1024:2048]
U_SH3 = 28                # 28, 29
U_SH2 = 30                # 30, 31: sh_w2 rows [0:1024], [1024:2048]
N_U8 = 32
# bf16 units in wall16
U_LIN1 = 0
U_S2A = 1
U_S2B = 2
N_U16 = 3

# bias-column groups in the const tensor [128, 7*KT]
BG_LIN0 = 0               # SH0 * lin0_b
BG_S1SG = 1               # swi1_b1 (true, sigmoid bias)
BG_S1A = 2                # SH0*SW**2... see _pack_biases
BG_S1B = 3
BG_LIN1 = 4               # true lin1_b
BG_S2A = 5                # true swi2_b1
BG_S2B = 6                # true swi2_b2

_prog = None  # built once per process
last_results = None  # BassKernelResults of the most recent kernel() call


def _build_program():
    import concourse.bacc as bacc
    import concourse.mybir as mybir
    import concourse.tile as tile

    F32, BF, F8 = mybir.dt.float32, mybir.dt.bfloat16, mybir.dt.float8e4
    AF = mybir.ActivationFunctionType
    OP = mybir.AluOpType
    DR = mybir.MatmulPerfMode.DoubleRow

    # descale constants (all exact powers of two)
    C_H0 = SH0 / (SX * SW)            # PSUM -> fp8 h0
    C_SG1 = 1.0 / (SH0 * SW)          # sigmoid input descale, swiglu1
    C_H = SHH / (SH0 * SW)            # f32 swiglu1 product -> fp8 h
    C_SGE = 1.0 / (SHH * SW)          # sigmoid input descale, experts/shared
    C_V = SV / (SHH * SW)             # expert/shared v -> fp8 (pre-combine)
    C_Y = 1.0 / (SV * SW)             # expert/shared y PSUM -> true-scale
    C_Z = 1.0 / (SHH * SW)            # router logits descale

    nc = bacc.Bacc()

    wall8_d = nc.dram_tensor("wall8", [N_U8 * 128, KT * 1024], F8,
                             kind="ExternalInput")
    wall16_d = nc.dram_tensor("wall16", [N_U16 * 128, KT * 1024], BF,
                              kind="ExternalInput")
    lin0n_d = nc.dram_tensor("lin0n", [128, KT * 1024], F8, kind="ExternalInput")
    xq_d = nc.dram_tensor("xq", [128, KT * TPC], F8, kind="ExternalInput")
    gq_d = nc.dram_tensor("gq", [128, KT * E], F8, kind="ExternalInput")
    bias_d = nc.dram_tensor("biases", [128, 7 * KT], F32, kind="ExternalInput")
    ident_d = nc.dram_tensor("ident", [128, 128], F32, kind="ExternalInput")
    sel_d = nc.dram_tensor("sel", [E, E * 128], F32, kind="ExternalInput")
    out_d = nc.dram_tensor("outT", [D, TPC], F32, kind="ExternalOutput")

    with tile.TileContext(nc) as tc, ExitStack() as ctx:
        wp = ctx.enter_context(tc.tile_pool(name="wp", bufs=6))
        sp = ctx.enter_context(tc.tile_pool(name="sp", bufs=1))
        dp = ctx.enter_context(tc.tile_pool(name="dp", bufs=4))
        pp = ctx.enter_context(tc.tile_pool(name="pp", bufs=2, space="PSUM"))

        def wload8(unit):
            wt = wp.tile([128, KT, 1024], F8, tag="wmat8", bufs=7, name=f"w8_{unit}")
            nc.sync.dma_start(
                wt[:],
                wall8_d[unit * 128:(unit + 1) * 128, :].rearrange(
                    "p (k c) -> p k c", k=KT),
            )
            return wt

        def wload16(unit):
            wt = wp.tile([128, KT, 1024], BF, tag="wmat16", bufs=3, name=f"w16_{unit}")
            nc.sync.dma_start(
                wt[:],
                wall16_d[unit * 128:(unit + 1) * 128, :].rearrange(
                    "p (k c) -> p k c", k=KT),
            )
            return wt

        # ---- static inputs ----
        xq = sp.tile([128, KT, TPC], F8, tag="xq", name="xq")
        nc.sync.dma_start(xq[:], xq_d[:].rearrange("p (k t) -> p k t", k=KT))
        gq = sp.tile([128, KT, E], F8, tag="gq", name="gq")
        nc.sync.dma_start(gq[:], gq_d[:].rearrange("p (k e) -> p k e", k=KT))
        biases = sp.tile([128, 7 * KT], F32, tag="biases", name="biases")
        nc.sync.dma_start(biases[:], bias_d[:])
        ident = sp.tile([128, 128], F32, tag="ident", name="ident")
        nc.sync.dma_start(ident[:], ident_d[:])
        sel = sp.tile([E, E * 128], F32, tag="sel", name="sel")
        nc.sync.dma_start(sel[:], sel_d[:])

        def bcol(idx, n):
            # per-partition bias column n of bias group idx
            return biases[:, idx * KT + n:idx * KT + n + 1]

        def mm8(ps, wt, src, n):
            # DoubleRow accumulation: ps += wt[:, :, n-block].T @ src
            for q in range(KT2):
                nc.tensor.matmul(ps[:], wt[:, 2 * q:2 * q + 2, n * 128:(n + 1) * 128],
                                 src[:, 2 * q:2 * q + 2, :],
                                 start=(q == 0), stop=(q == KT2 - 1), perf_mode=DR)

        # ---- block 1: h0 = x @ lin0 + b (fp8 in, fp8 out) ----
        # lin0 arrives as 8 contiguous column-chunks (n-major host pack) so
        # the n=0 matmul starts after 128KB instead of 1MB of DMA
        w_lin0 = wp.tile([128, KT, KT, 128], F8, tag="wmat8", bufs=7,
                         name="w8_lin0")
        lin0_src = lin0n_d[:].rearrange("p (n k c) -> p n k c", n=KT, k=KT)
        for n in range(KT):
            nc.sync.dma_start(w_lin0[:, n, :, :], lin0_src[:, n, :, :])
        h0q = sp.tile([128, KT, TPC], F8, tag="h0q", name="h0q")
        for n in range(KT):
            ps = pp.tile([128, TPC], F32, tag="gu", bufs=4, name="ps_h0")
            for q in range(KT2):
                nc.tensor.matmul(ps[:], w_lin0[:, n, 2 * q:2 * q + 2, :],
                                 xq[:, 2 * q:2 * q + 2, :],
                                 start=(q == 0), stop=(q == KT2 - 1), perf_mode=DR)
            nc.scalar.activation(h0q[:, n, :], ps[:], AF.Identity,
                                 bias=bcol(BG_LIN0, n), scale=C_H0)

        # ---- swiglu 1 -> h (the MoE input), fp8 ----
        w_s1a, w_s1b = wload8(U_S1A), wload8(U_S1B)
        hq = sp.tile([128, KT, TPC], F8, tag="hq", name="hq")
        for m in range(KT):
            halves = (((0, TPC),) if m < KT - 1 else
                      ((0, TPC // 2), (TPC // 2, TPC)))
            for lo, hi in halves:
                pa = pp.tile([128, hi - lo], F32, tag="gu", bufs=4, name="ps_a1")
                for q in range(KT2):
                    nc.tensor.matmul(
                        pa[:], w_s1a[:, 2 * q:2 * q + 2, m * 128:(m + 1) * 128],
                        h0q[:, 2 * q:2 * q + 2, lo:hi],
                        start=(q == 0), stop=(q == KT2 - 1), perf_mode=DR)
                pb = pp.tile([128, hi - lo], F32, tag="gu", bufs=4, name="ps_b1")
                for q in range(KT2):
                    nc.tensor.matmul(
                        pb[:], w_s1b[:, 2 * q:2 * q + 2, m * 128:(m + 1) * 128],
                        h0q[:, 2 * q:2 * q + 2, lo:hi],
                        start=(q == 0), stop=(q == KT2 - 1), perf_mode=DR)
                sg = dp.tile([128, hi - lo], F32, tag="gs", bufs=3, name="sg1")
                nc.scalar.activation(sg[:], pa[:], AF.Silu,
                                     bias=bcol(BG_S1SG, m), scale=C_SG1)
                t = dp.tile([128, hi - lo], F32, tag="v", bufs=3, name="t1")
                nc.vector.scalar_tensor_tensor(t[:], pb[:], bcol(BG_S1B, m),
                                               sg[:], OP.add, OP.mult)
                nc.scalar.activation(hq[:, m, lo:hi], t[:], AF.Copy, scale=C_H)

        # ---- router matmuls: z[t, e] for all 4 token sub-tiles ----
        z_all = pp.tile([128, NT * E], F32, tag="misc", bufs=2, name="z_all")
        for t in range(NT):
            for k in range(KT):
                nc.tensor.matmul(z_all[:, t * E:(t + 1) * E],
                                 hq[:, k, t * 128:(t + 1) * 128],
                                 gq[:, k, :], start=(k == 0), stop=(k == KT - 1))

        # ---- router chain (DVE/ACT; overlaps expert-0 g/u matmuls on PE) ----
        ez = sp.tile([128, NT * E], F32, tag="ez", name="ez")
        cur = sp.tile([128, NT * E], F32, tag="cur", name="cur")
        cm = sp.tile([128, NT * E], F32, tag="cm", name="cm")
        combine = sp.tile([128, NT * E], F32, tag="combine", name="combine")
        stat = sp.tile([128, 4 * NT], F32, tag="stat", name="stat")  # nmx, thr, s, r

        for t in range(NT):
            zt = z_all[:, t * E:(t + 1) * E]
            nmx = stat[:, t:t + 1]
            nc.vector.tensor_reduce(nmx, zt, mybir.AxisListType.X, OP.max, negate=True)
            # nmx holds -max of the scaled logits; Exp gets scale applied to
            # the input only, so pre-scale the bias to true units
            nc.vector.tensor_scalar(nmx, nmx, C_Z, None, OP.mult)
            ezt = ez[:, t * E:(t + 1) * E]
            nc.scalar.activation(ezt, zt, AF.Exp, bias=nmx, scale=C_Z)
            curt = cur[:, t * E:(t + 1) * E]
            nc.vector.tensor_copy(curt, ezt)
            thr = stat[:, NT + t:NT + t + 1]
            for i in range(TOPK):
                nc.vector.tensor_reduce(thr, curt, mybir.AxisListType.X, OP.max)
                if i < TOPK - 1:
                    eq = dp.tile([128, E], F32, tag="eq", bufs=2, name="eq")
                    nc.vector.tensor_scalar(eq[:], curt, thr, None, OP.is_equal)
                    nc.vector.scalar_tensor_tensor(curt, eq[:], -1e30, curt,
                                                   OP.mult, OP.add)
            cmt = cm[:, t * E:(t + 1) * E]
            # cm = ez * (ez >= thr); reuse cur as the mask buffer
            nc.vector.tensor_scalar(curt, ezt, thr, None, OP.is_ge)
            nc.vector.tensor_mul(cmt, ezt, curt)
            s = stat[:, 2 * NT + t:2 * NT + t + 1]
            nc.vector.tensor_reduce(s, cmt, mybir.AxisListType.X, OP.add)
            r = stat[:, 3 * NT + t:3 * NT + t + 1]
            nc.vector.reciprocal(r, s)
            nc.vector.tensor_scalar(combine[:, t * E:(t + 1) * E], cmt, r, None,
                                    OP.mult)

        cbT = sp.tile([E, TPC], F32, tag="cbT", name="cbT")

        def emit_transposes():
            for t in range(NT):
                trp = pp.tile([E, 128], F32, tag="misc", name="trp")
                nc.tensor.transpose(trp[:], combine[:, t * E:(t + 1) * E], ident[:])
                nc.scalar.activation(cbT[0:E, t * 128:(t + 1) * 128], trp[:], AF.Copy)

        def outer(e):
            # cb_ps[p, t] = sum_k sel[k, e*128+p] * cbT[k, t] = combine[t, e]
            cb_ps = pp.tile([128, TPC], F32, tag="misc", name="cb_ps")
            nc.tensor.matmul(cb_ps[:], sel[:, e * 128:(e + 1) * 128], cbT[0:E, :],
                             start=True, stop=True)
            return cb_ps

        def emit_gu(w1, w3, n_m, cb_ps, tag):
            """g/u/v for one expert (n_m m-tiles) -> fp8 vb tile
            [128, n_m, TPC], scaled by cb_ps when given."""
            vb = dp.tile([128, n_m, TPC], F8, tag=tag, bufs=(3 if n_m == MT_H else 1), name="vb")
            for m in range(n_m):
                u, mm = divmod(m, KT)
                pg = pp.tile([128, TPC], F32, tag="gu", bufs=4, name="ps_g")
                mm8(pg, w1[u], hq, mm)
                pu = pp.tile([128, TPC], F32, tag="gu", bufs=4, name="ps_u")
                mm8(pu, w3[u], hq, mm)
                sg = dp.tile([128, TPC], F32, tag="gs", bufs=3, name="sg")
                nc.scalar.activation(sg[:], pg[:], AF.Silu, scale=C_SGE)
                if cb_ps is None:
                    nc.vector.scalar_tensor_tensor(vb[:, m, :], pu[:], C_V, sg[:],
                                                   OP.mult, OP.mult)
                else:
                    v = dp.tile([128, TPC], F32, tag="v", bufs=3, name="v")
                    nc.vector.scalar_tensor_tensor(v[:], pu[:], C_V, sg[:],
                                                   OP.mult, OP.mult)
                    nc.vector.tensor_mul(vb[:, m, :], v[:], cb_ps[:])
            return vb

        def emit_y(w2, vb, n_m, acc, cb_sb=None, final=None):
            """y = vb @ w2 accumulated into acc (fp32 SBUF). w2: list of units.
            final: list to receive bf16 direct-out tiles (last stage)."""
            for n in range(KT):
                py = pp.tile([128, TPC], F32, tag="y", bufs=2, name="ps_y")
                for q in range(n_m // 2):
                    u, qq = divmod(q, KT2)
                    nc.tensor.matmul(py[:],
                                     w2[u][:, 2 * qq:2 * qq + 2, n * 128:(n + 1) * 128],
                                     vb[:, 2 * q:2 * q + 2, :],
                                     start=(q == 0), stop=(q == n_m // 2 - 1),
                                     perf_mode=DR)
                if cb_sb is not None:
                    a = sp.tile([128, TPC], F32, tag="acc", bufs=8, name=f"acc_{n}")
                    nc.vector.tensor_mul(a[:], py[:], cb_sb[:])
                    acc.append(a)
                elif final is not None:
                    t = sp.tile([128, TPC], BF, tag="accbf", bufs=8,
                                name=f"accbf_{n}")
                    nc.vector.scalar_tensor_tensor(t[:], py[:], C_Y, acc[n][:],
                                                   OP.mult, OP.add)
                    final.append(t)
                else:
                    nc.vector.scalar_tensor_tensor(acc[n][:], py[:], C_Y, acc[n][:],
                                                   OP.mult, OP.add)

        # ---- experts (fp8; expert 0 applies combine on the output side,
        # the rest fold it into vb before the w2 matmul) ----
        acc = []
        vb_prev = w2_prev = cb_sb0 = None
        for e in range(E):
            we1 = wload8(U_EXP + 3 * e)
            we3 = wload8(U_EXP + 3 * e + 1)
            we2 = wload8(U_EXP + 3 * e + 2)
            if e == 0:
                vb = emit_gu([we1], [we3], MT_H, None, "vb")
                emit_transposes()
                cb_ps0 = outer(0)
                cb_sb0 = dp.tile([128, TPC], F32, tag="cbsb", bufs=1, name="cb_sb0")
                # descale C_Y folded in: acc = py * (combine * C_Y)
                nc.scalar.activation(cb_sb0[:], cb_ps0[:], AF.Copy, scale=C_Y)
            else:
                cb_ps = outer(e)
                vb = emit_gu([we1], [we3], MT_H, cb_ps, "vb")
                emit_y([w2_prev], vb_prev, MT_H, acc,
                       cb_sb=(cb_sb0 if e == 1 else None))
            vb_prev, w2_prev = vb, we2

        # ---- shared expert (always-on, unscaled) ----
        sh1 = [wload8(U_SH1), wload8(U_SH1 + 1)]
        sh3 = [wload8(U_SH3), wload8(U_SH3 + 1)]
        sh2 = [wload8(U_SH2), wload8(U_SH2 + 1)]
        emit_y([w2_prev], vb_prev, MT_H, acc)
        vbsh = emit_gu(sh1, sh3, MT_SH, None, "vbsh")
        w_lin1 = wload16(U_LIN1)
        w_s2a = wload16(U_S2A)
        w_s2b = wload16(U_S2B)
        accbf = []
        emit_y(sh2, vbsh, MT_SH, acc, final=accbf)

        # ---- block 3: lin1 + swiglu2 (bf16, true scale) ----

        h2 = []
        for n in range(KT):
            ps = pp.tile([128, TPC], F32, tag="gu", bufs=4, name="ps_h2")
            for k in range(KT):
                nc.tensor.matmul(ps[:], w_lin1[:, k, n * 128:(n + 1) * 128],
                                 accbf[k][:], start=(k == 0), stop=(k == KT - 1))
            t = sp.tile([128, TPC], BF, tag="h2", bufs=8, name=f"h2_{n}")
            nc.scalar.activation(t[:], ps[:], AF.Identity, bias=bcol(BG_LIN1, n))
            h2.append(t)

        for m in range(KT):
            halves = (((0, TPC),) if m < KT - 1 else
                      ((0, TPC // 2), (TPC // 2, TPC)))
            for lo, hi in halves:
                pa = pp.tile([128, hi - lo], F32, tag="gu", bufs=4, name="ps_a2")
                for k in range(KT):
                    nc.tensor.matmul(pa[:], w_s2a[:, k, m * 128:(m + 1) * 128],
                                     h2[k][:, lo:hi], start=(k == 0),
                                     stop=(k == KT - 1))
                pb = pp.tile([128, hi - lo], F32, tag="gu", bufs=4, name="ps_b2")
                for k in range(KT):
                    nc.tensor.matmul(pb[:], w_s2b[:, k, m * 128:(m + 1) * 128],
                                     h2[k][:, lo:hi], start=(k == 0),
                                     stop=(k == KT - 1))
                sg = dp.tile([128, hi - lo], F32, tag="gs", bufs=3, name="sg2")
                nc.scalar.activation(sg[:], pa[:], AF.Silu, bias=bcol(BG_S2A, m))
                o = dp.tile([128, hi - lo], F32, tag="out", bufs=2, name="o")
                nc.vector.scalar_tensor_tensor(o[:], pb[:], bcol(BG_S2B, m),
                                               sg[:], OP.add, OP.mult)
                nc.sync.dma_start(out_d[m * 128:(m + 1) * 128, lo:hi], o[:])

    # run_bass_via_pjrt serializes the BIR as-is; Bacc's lowering passes
    # (register allocation, TRN2 single-wait splitting) only run in
    # finalize(), so it must happen before dispatch.
    nc.finalize()
    return nc


def _pack8(w):
    """[1024, 1024] f32 -> [128, KT*1024] e4m3 rows, p-major contiguous."""
    q = np.clip(np.asarray(w, np.float32) * SW, -240, 240).astype(f8)
    return np.ascontiguousarray(
        q.reshape(KT, 128, 1024).transpose(1, 0, 2).reshape(128, KT * 1024))


def _pack16(w):
    q = np.asarray(w, np.float32).astype(bf16)
    return np.ascontiguousarray(
        q.reshape(KT, 128, 1024).transpose(1, 0, 2).reshape(128, KT * 1024))


def _pack_lin0n(w):
    """[1024, 1024] -> [128, KT*KT*128] e4m3, n-major then k then c."""
    q = np.clip(np.asarray(w, np.float32) * SW, -240, 240).astype(f8)
    # q[k*128+p, n*128+c] -> out[p, ((n*KT)+k)*128+c]
    q = q.reshape(KT, 128, KT, 128).transpose(1, 2, 0, 3)  # p, n, k, c
    return np.ascontiguousarray(q.reshape(128, KT * KT * 128))


def _pack_weights(inp):
    units = [_pack8(inp["swi1_w1"]), _pack8(inp["swi1_w2"])]
    w1, w3, w2 = (np.asarray(inp["exp_w1"], np.float32),
                  np.asarray(inp["exp_w3"], np.float32),
                  np.asarray(inp["exp_w2"], np.float32))
    for e in range(E):
        units += [_pack8(w1[e]), _pack8(w3[e]), _pack8(w2[e])]
    sh1 = np.asarray(inp["sh_w1"], np.float32)
    sh3 = np.asarray(inp["sh_w3"], np.float32)
    sh2 = np.asarray(inp["sh_w2"], np.float32)
    units += [_pack8(sh1[:, :1024]), _pack8(sh1[:, 1024:]),
              _pack8(sh3[:, :1024]), _pack8(sh3[:, 1024:]),
              _pack8(sh2[:1024, :]), _pack8(sh2[1024:, :])]
    assert len(units) == N_U8
    wall8 = np.ascontiguousarray(np.concatenate(units, axis=0))
    u16 = [_pack16(inp["lin1_w"]), _pack16(inp["swi2_w1"]), _pack16(inp["swi2_w2"])]
    wall16 = np.ascontiguousarray(np.concatenate(u16, axis=0))
    return wall8, wall16


def _pack_biases(inp):
    scaled = [
        ("lin0_b", SH0),
        ("swi1_b1", 1.0),
        ("swi1_b1", SH0 * SW),
        ("swi1_b2", SH0 * SW),
        ("lin1_b", 1.0),
        ("swi2_b1", 1.0),
        ("swi2_b2", 1.0),
    ]
    cols = []
    for name, s in scaled:
        v = (np.asarray(inp[name], np.float32) * s).reshape(KT, 128).T
        cols.append(v)
    return np.ascontiguousarray(np.concatenate(cols, axis=1))  # [128, 7*KT]


def kernel(**inputs):
    global _prog
    from concourse.bass_utils import run_bass_kernel_spmd

    if _prog is None:
        _prog = _build_program()
    nc = _prog

    wall8, wall16 = _pack_weights(inputs)
    lin0n = _pack_lin0n(inputs["lin0_w"])
    biases = _pack_biases(inputs)
    gq = np.clip(np.asarray(inputs["gate_w"], np.float32).T * SW, -240, 240).astype(f8)
    gq = np.ascontiguousarray(
        gq.reshape(KT, 128, E).transpose(1, 0, 2).reshape(128, KT * E))
    ident = np.eye(128, dtype=np.float32)
    sel = np.zeros((E, E * 128), dtype=np.float32)
    for e in range(E):
        sel[e, e * 128:(e + 1) * 128] = 1.0

    x = np.asarray(inputs["x"], np.float32).reshape(T, D)
    in_maps = []
    for c in range(NCORES):
        xT = np.clip(x[c * TPC:(c + 1) * TPC, :].T * SX, -240, 240).astype(f8)
        xqc = np.ascontiguousarray(
            xT.reshape(KT, 128, TPC).transpose(1, 0, 2).reshape(128, KT * TPC))
        in_maps.append({
            "wall8": wall8, "wall16": wall16, "lin0n": lin0n, "xq": xqc,
            "gq": gq, "biases": biases, "ident": ident, "sel": sel,
        })

    res = run_bass_kernel_spmd(nc, in_maps, list(range(NCORES)))
    global last_results
    last_results = res
    outT = np.concatenate([res.results[c]["outT"] for c in range(NCORES)], axis=1)
    return np.ascontiguousarray(outT.T).reshape(B, S, D).astype(np.float32)


# revision 16
# speedup vs baseline: 1.1852x; 1.1852x over previous
"""Trainium2 Bass kernel for the MoE feed-forward block (nn_MoEFF).

Strategy: pure data-parallel over tokens. The 4096 tokens are split into
8 slices of 512; each NeuronCore runs the *entire* network on its slice
(router + all 8 experts dense-masked + shared expert). With E=8/K=4 every
expert serves ~half the tokens anyway, so dense-masked expert compute
costs only 2x the ideal sparse flops and avoids all collectives.

Precision: the front block (lin0 + swiglu1), router, all routed experts
and the shared expert run in fp8 e4m3 with DoubleRow matmuls (2 fp8
weights per PE cell, 256-deep contraction per instruction -> ~1.76x
bf16 matmul throughput measured). Activations and weights carry static
power-of-2 scales chosen so sigma*scale ~ 8-20 (TRN e4m3 max normal is
240; every descale constant folds into an existing activation/vector
op). The back block (lin1 + swiglu2) stays bf16: fp8 noise there lands
directly on the output with no averaging downstream, and it is only
3/36 of the matmul work. Measured end-to-end rel err 7.1e-3.

Layout: activations are kept transposed in SBUF ([feature-on-partition,
token-on-free]) so weight matrices in natural [in, out] layout are the
stationary matmul operand. Weights are host-packed p-major so every
DMA row is contiguous; lin0 additionally ships n-major so its first
column chunk (and the first matmul) starts ~4us earlier. Each expert's
down-projection is emitted one expert late so its PSUM->fp8 vector
chain hides under the next expert's up-projection matmuls. All PSUM
accumulation is fp32.
"""

from contextlib import ExitStack

import ml_dtypes
import numpy as np

B, S, D = 2, 2048, 1024
E, TOPK, H = 8, 4, 1024
SH = 2 * H
NCORES = 8
T = B * S                 # 4096 tokens
TPC = T // NCORES         # 512 tokens per core
KT = D // 128             # 8 contraction tiles
KT2 = KT // 2             # 4 DoubleRow contraction steps
MT_H = H // 128           # 8
MT_SH = SH // 128         # 16
NT = TPC // 128           # 4 token sub-tiles (router)

bf16 = ml_dtypes.bfloat16
f8 = ml_dtypes.float8_e4m3  # TRN FP8_EXP4-compatible (max normal 240)

# static power-of-2 scales (data ~N(0, sigma) with known sigmas)
SW = 512.0                # fp8 weights (sigma 0.02 -> 10)
SX = 16.0                 # x (sigma 1.0)
SH0 = 32.0                # h0 (sigma 0.64)
SHH = 64.0                # h, the MoE input (sigma 0.096, absmax 2.24)
SV = 4096.0               # expert/shared v = silu(g)*u (sigma 0.002)

# fp8 unit layout in wall8 (each unit is one [1024, 1024] matrix,
# host-packed to [128, KT*1024] p-major rows)
U_S1A = 0
U_S1B = 1
U_EXP = 2                 # 2 + 3*e + {0: w1, 1: w3, 2: w2}
U_SH1 = 26                # 26, 27: sh_w1 cols [0:1024], [1024:2048]
U_SH3 = 28                # 28, 29
U_SH2 = 30                # 30, 31: sh_w2 rows [0:1024], [1024:2048]
N_U8 = 32
# bf16 units in wall16
U_LIN1 = 0
U_S2A = 1
U_S2B = 2
N_U16 = 3

# bias-column groups in the const tensor [128, 7*KT]
BG_LIN0 = 0               # SH0 * lin0_b
BG_S1SG = 1               # swi1_b1 (true, sigmoid bias)
BG_S1A = 2                # SH0*SW**2... see _pack_biases
BG_S1B = 3
BG_LIN1 = 4               # true lin1_b
BG_S2A = 5                # true swi2_b1
BG_S2B = 6                # true swi2_b2

_prog = None  # built once per process
last_results = None  # BassKernelResults of the most recent kernel() call


def _build_program():
    import concourse.bacc as bacc
    import concourse.mybir as mybir
    import concourse.tile as tile

    F32, BF, F8 = mybir.dt.float32, mybir.dt.bfloat16, mybir.dt.float8e4
    AF = mybir.ActivationFunctionType
    OP = mybir.AluOpType
    DR = mybir.MatmulPerfMode.DoubleRow

    # descale constants (all exact powers of two)
    C_H0 = SH0 / (SX * SW)            # PSUM -> fp8 h0
    C_SG1 = 1.0 / (SH0 * SW)          # sigmoid input descale, swiglu1
    C_H = SHH / (SH0 * SW)            # f32 swiglu1 product -> fp8 h
    C_SGE = 1.0 / (SHH * SW)          # sigmoid input descale, experts/shared
    C_V = SV / (SHH * SW)             # expert/shared v -> fp8 (pre-combine)
    C_Y = 1.0 / (SV * SW)             # expert/shared y PSUM -> true-scale
    C_Z = 1.0 / (SHH * SW)            # router logits descale

    nc = bacc.Bacc()

    wall8_d = nc.dram_tensor("wall8", [N_U8 * 128, KT * 1024], F8,
                             kind="ExternalInput")
    wall16_d = nc.dram_tensor("wall16", [N_U16 * 128, KT * 1024], BF,
                              kind="ExternalInput")
    lin0n_d = nc.dram_tensor("lin0n", [128, KT * 1024], F8, kind="ExternalInput")
    xq_d = nc.dram_tensor("xq", [128, KT * TPC], F8, kind="ExternalInput")
    gq_d = nc.dram_tensor("gq", [128, KT * E], F8, kind="ExternalInput")
    bias_d = nc.dram_tensor("biases", [128, 7 * KT], F32, kind="ExternalInput")
    ident_d = nc.dram_tensor("ident", [128, 128], F32, kind="ExternalInput")
    sel_d = nc.dram_tensor("sel", [E, E * 128], F32, kind="ExternalInput")
    out_d = nc.dram_tensor("outT", [D, TPC], F32, kind="ExternalOutput")

    with tile.TileContext(nc) as tc, ExitStack() as ctx:
        wp = ctx.enter_context(tc.tile_pool(name="wp", bufs=6))
        sp = ctx.enter_context(tc.tile_pool(name="sp", bufs=1))
        dp = ctx.enter_context(tc.tile_pool(name="dp", bufs=4))
        pp = ctx.enter_context(tc.tile_pool(name="pp", bufs=2, space="PSUM"))

        def wload8(unit):
            wt = wp.tile([128, KT, 1024], F8, tag="wmat8", bufs=7, name=f"w8_{unit}")
            nc.sync.dma_start(
                wt[:],
                wall8_d[unit * 128:(unit + 1) * 128, :].rearrange(
                    "p (k c) -> p k c", k=KT),
            )
            return wt

        def wload16(unit):
            wt = wp.tile([128, KT, 1024], BF, tag="wmat16", bufs=3, name=f"w16_{unit}")
            nc.sync.dma_start(
                wt[:],
                wall16_d[unit * 128:(unit + 1) * 128, :].rearrange(
                    "p (k c) -> p k c", k=KT),
            )
            return wt

        # ---- static inputs ----
        xq = sp.tile([128, KT, TPC], F8, tag="xq", name="xq")
        nc.sync.dma_start(xq[:], xq_d[:].rearrange("p (k t) -> p k t", k=KT))
        gq = sp.tile([128, KT, E], F8, tag="gq", name="gq")
        nc.sync.dma_start(gq[:], gq_d[:].rearrange("p (k e) -> p k e", k=KT))
        biases = sp.tile([128, 7 * KT], F32, tag="biases", name="biases")
        nc.sync.dma_start(biases[:], bias_d[:])
        ident = sp.tile([128, 128], F32, tag="ident", name="ident")
        nc.sync.dma_start(ident[:], ident_d[:])
        sel = sp.tile([E, E * 128], F32, tag="sel", name="sel")
        nc.sync.dma_start(sel[:], sel_d[:])

        def bcol(idx, n):
            # per-partition bias column n of bias group idx
            return biases[:, idx * KT + n:idx * KT + n + 1]

        def mm8(ps, wt, src, n):
            # DoubleRow accumulation: ps += wt[:, :, n-block].T @ src
            for q in range(KT2):
                nc.tensor.matmul(ps[:], wt[:, 2 * q:2 * q + 2, n * 128:(n + 1) * 128],
                                 src[:, 2 * q:2 * q + 2, :],
                                 start=(q == 0), stop=(q == KT2 - 1), perf_mode=DR)

        # ---- block 1: h0 = x @ lin0 + b (fp8 in, fp8 out) ----
        # lin0 arrives as 8 contiguous column-chunks (n-major host pack) so
        # the n=0 matmul starts after 128KB instead of 1MB of DMA
        w_lin0 = wp.tile([128, KT, KT, 128], F8, tag="wmat8", bufs=7,
                         name="w8_lin0")
        lin0_src = lin0n_d[:].rearrange("p (n k c) -> p n k c", n=KT, k=KT)
        for n in range(KT):
            nc.sync.dma_start(w_lin0[:, n, :, :], lin0_src[:, n, :, :])
        h0q = sp.tile([128, KT, TPC], F8, tag="h0q", name="h0q")
        for n in range(KT):
            ps = pp.tile([128, TPC], F32, tag="gu", bufs=4, name="ps_h0")
            for q in range(KT2):
                nc.tensor.matmul(ps[:], w_lin0[:, n, 2 * q:2 * q + 2, :],
                                 xq[:, 2 * q:2 * q + 2, :],
                                 start=(q == 0), stop=(q == KT2 - 1), perf_mode=DR)
            nc.scalar.activation(h0q[:, n, :], ps[:], AF.Identity,
                                 bias=bcol(BG_LIN0, n), scale=C_H0)

        # ---- swiglu 1 -> h (the MoE input), fp8 ----
        w_s1a, w_s1b = wload8(U_S1A), wload8(U_S1B)
        hq = sp.tile([128, KT, TPC], F8, tag="hq", name="hq")
        for m in range(KT):
            pa = pp.tile([128, TPC], F32, tag="gu", bufs=4, name="ps_a1")
            mm8(pa, w_s1a, h0q, m)
            pb = pp.tile([128, TPC], F32, tag="gu", bufs=4, name="ps_b1")
            mm8(pb, w_s1b, h0q, m)
            sg = dp.tile([128, TPC], F32, tag="gs", bufs=3, name="sg1")
            nc.scalar.activation(sg[:], pa[:], AF.Silu,
                                 bias=bcol(BG_S1SG, m), scale=C_SG1)
            t = dp.tile([128, TPC], F32, tag="v", bufs=3, name="t1")
            nc.vector.scalar_tensor_tensor(t[:], pb[:], bcol(BG_S1B, m), sg[:],
                                           OP.add, OP.mult)
            nc.scalar.activation(hq[:, m, :], t[:], AF.Copy, scale=C_H)

        # ---- router matmuls: z[t, e] for all 4 token sub-tiles ----
        z_all = pp.tile([128, NT * E], F32, tag="misc", bufs=2, name="z_all")
        for t in range(NT):
            for k in range(KT):
                nc.tensor.matmul(z_all[:, t * E:(t + 1) * E],
                                 hq[:, k, t * 128:(t + 1) * 128],
                                 gq[:, k, :], start=(k == 0), stop=(k == KT - 1))

        # ---- router chain (DVE/ACT; overlaps expert-0 g/u matmuls on PE) ----
        ez = sp.tile([128, NT * E], F32, tag="ez", name="ez")
        cur = sp.tile([128, NT * E], F32, tag="cur", name="cur")
        cm = sp.tile([128, NT * E], F32, tag="cm", name="cm")
        combine = sp.tile([128, NT * E], F32, tag="combine", name="combine")
        stat = sp.tile([128, 4 * NT], F32, tag="stat", name="stat")  # nmx, thr, s, r

        for t in range(NT):
            zt = z_all[:, t * E:(t + 1) * E]
            nmx = stat[:, t:t + 1]
            nc.vector.tensor_reduce(nmx, zt, mybir.AxisListType.X, OP.max, negate=True)
            # nmx holds -max of the scaled logits; Exp gets scale applied to
            # the input only, so pre-scale the bias to true units
            nc.vector.tensor_scalar(nmx, nmx, C_Z, None, OP.mult)
            ezt = ez[:, t * E:(t + 1) * E]
            nc.scalar.activation(ezt, zt, AF.Exp, bias=nmx, scale=C_Z)
            curt = cur[:, t * E:(t + 1) * E]
            nc.vector.tensor_copy(curt, ezt)
            thr = stat[:, NT + t:NT + t + 1]
            for i in range(TOPK):
                nc.vector.tensor_reduce(thr, curt, mybir.AxisListType.X, OP.max)
                if i < TOPK - 1:
                    eq = dp.tile([128, E], F32, tag="eq", bufs=2, name="eq")
                    nc.vector.tensor_scalar(eq[:], curt, thr, None, OP.is_equal)
                    nc.vector.scalar_tensor_tensor(curt, eq[:], -1e30, curt,
                                                   OP.mult, OP.add)
            cmt = cm[:, t * E:(t + 1) * E]
            # cm = ez * (ez >= thr); reuse cur as the mask buffer
            nc.vector.tensor_scalar(curt, ezt, thr, None, OP.is_ge)
            nc.vector.tensor_mul(cmt, ezt, curt)
            s = stat[:, 2 * NT + t:2 * NT + t + 1]
            nc.vector.tensor_reduce(s, cmt, mybir.AxisListType.X, OP.add)
            r = stat[:, 3 * NT + t:3 * NT + t + 1]
            nc.vector.reciprocal(r, s)
            nc.vector.tensor_scalar(combine[:, t * E:(t + 1) * E], cmt, r, None,
                                    OP.mult)

        cbT = sp.tile([E, TPC], F32, tag="cbT", name="cbT")

        def emit_transposes():
            for t in range(NT):
                trp = pp.tile([E, 128], F32, tag="misc", name="trp")
                nc.tensor.transpose(trp[:], combine[:, t * E:(t + 1) * E], ident[:])
                nc.scalar.activation(cbT[0:E, t * 128:(t + 1) * 128], trp[:], AF.Copy)

        def outer(e):
            # cb_ps[p, t] = sum_k sel[k, e*128+p] * cbT[k, t] = combine[t, e]
            cb_ps = pp.tile([128, TPC], F32, tag="misc", name="cb_ps")
            nc.tensor.matmul(cb_ps[:], sel[:, e * 128:(e + 1) * 128], cbT[0:E, :],
                             start=True, stop=True)
            return cb_ps

        def emit_gu(w1, w3, n_m, cb_ps, tag):
            """g/u/v for one expert (n_m m-tiles) -> fp8 vb tile
            [128, n_m, TPC], scaled by cb_ps when given."""
            vb = dp.tile([128, n_m, TPC], F8, tag=tag, bufs=(3 if n_m == MT_H else 1), name="vb")
            for m in range(n_m):
                u, mm = divmod(m, KT)
                pg = pp.tile([128, TPC], F32, tag="gu", bufs=4, name="ps_g")
                mm8(pg, w1[u], hq, mm)
                pu = pp.tile([128, TPC], F32, tag="gu", bufs=4, name="ps_u")
                mm8(pu, w3[u], hq, mm)
                sg = dp.tile([128, TPC], F32, tag="gs", bufs=3, name="sg")
                nc.scalar.activation(sg[:], pg[:], AF.Silu, scale=C_SGE)
                if cb_ps is None:
                    nc.vector.scalar_tensor_tensor(vb[:, m, :], pu[:], C_V, sg[:],
                                                   OP.mult, OP.mult)
                else:
                    v = dp.tile([128, TPC], F32, tag="v", bufs=3, name="v")
                    nc.vector.scalar_tensor_tensor(v[:], pu[:], C_V, sg[:],
                                                   OP.mult, OP.mult)
                    nc.vector.tensor_mul(vb[:, m, :], v[:], cb_ps[:])
            return vb

        def emit_y(w2, vb, n_m, acc, cb_sb=None, final=None):
            """y = vb @ w2 accumulated into acc (fp32 SBUF). w2: list of units.
            final: list to receive bf16 direct-out tiles (last stage)."""
            for n in range(KT):
                py = pp.tile([128, TPC], F32, tag="y", bufs=2, name="ps_y")
                for q in range(n_m // 2):
                    u, qq = divmod(q, KT2)
                    nc.tensor.matmul(py[:],
                                     w2[u][:, 2 * qq:2 * qq + 2, n * 128:(n + 1) * 128],
                                     vb[:, 2 * q:2 * q + 2, :],
                                     start=(q == 0), stop=(q == n_m // 2 - 1),
                                     perf_mode=DR)
                if cb_sb is not None:
                    a = sp.tile([128, TPC], F32, tag="acc", bufs=8, name=f"acc_{n}")
                    nc.vector.tensor_mul(a[:], py[:], cb_sb[:])
                    acc.append(a)
                elif final is not None:
                    t = sp.tile([128, TPC], BF, tag="accbf", bufs=8,
                                name=f"accbf_{n}")
                    nc.vector.scalar_tensor_tensor(t[:], py[:], C_Y, acc[n][:],
                                                   OP.mult, OP.add)
                    final.append(t)
                else:
                    nc.vector.scalar_tensor_tensor(acc[n][:], py[:], C_Y, acc[n][:],
                                                   OP.mult, OP.add)

        # ---- experts (fp8; expert 0 applies combine on the output side,
        # the rest fold it into vb before the w2 matmul) ----
        acc = []
        vb_prev = w2_prev = cb_sb0 = None
        for e in range(E):
            we1 = wload8(U_EXP + 3 * e)
            we3 = wload8(U_EXP + 3 * e + 1)
            we2 = wload8(U_EXP + 3 * e + 2)
            if e == 0:
                vb = emit_gu([we1], [we3], MT_H, None, "vb")
                emit_transposes()
                cb_ps0 = outer(0)
                cb_sb0 = dp.tile([128, TPC], F32, tag="cbsb", bufs=1, name="cb_sb0")
                # descale C_Y folded in: acc = py * (combine * C_Y)
                nc.scalar.activation(cb_sb0[:], cb_ps0[:], AF.Copy, scale=C_Y)
            else:
                cb_ps = outer(e)
                vb = emit_gu([we1], [we3], MT_H, cb_ps, "vb")
                emit_y([w2_prev], vb_prev, MT_H, acc,
                       cb_sb=(cb_sb0 if e == 1 else None))
            vb_prev, w2_prev = vb, we2

        # ---- shared expert (always-on, unscaled) ----
        sh1 = [wload8(U_SH1), wload8(U_SH1 + 1)]
        sh3 = [wload8(U_SH3), wload8(U_SH3 + 1)]
        sh2 = [wload8(U_SH2), wload8(U_SH2 + 1)]
        emit_y([w2_prev], vb_prev, MT_H, acc)
        vbsh = emit_gu(sh1, sh3, MT_SH, None, "vbsh")
        w_lin1 = wload16(U_LIN1)
        w_s2a = wload16(U_S2A)
        w_s2b = wload16(U_S2B)
        accbf = []
        emit_y(sh2, vbsh, MT_SH, acc, final=accbf)

        # ---- block 3: lin1 + swiglu2 (bf16, true scale) ----

        h2 = []
        for n in range(KT):
            ps = pp.tile([128, TPC], F32, tag="gu", bufs=4, name="ps_h2")
            for k in range(KT):
                nc.tensor.matmul(ps[:], w_lin1[:, k, n * 128:(n + 1) * 128],
                                 accbf[k][:], start=(k == 0), stop=(k == KT - 1))
            t = sp.tile([128, TPC], BF, tag="h2", bufs=8, name=f"h2_{n}")
            nc.scalar.activation(t[:], ps[:], AF.Identity, bias=bcol(BG_LIN1, n))
            h2.append(t)

        for m in range(KT):
            pa = pp.tile([128, TPC], F32, tag="gu", bufs=4, name="ps_a2")
            for k in range(KT):
                nc.tensor.matmul(pa[:], w_s2a[:, k, m * 128:(m + 1) * 128],
                                 h2[k][:], start=(k == 0), stop=(k == KT - 1))
            pb = pp.tile([128, TPC], F32, tag="gu", bufs=4, name="ps_b2")
            for k in range(KT):
                nc.tensor.matmul(pb[:], w_s2b[:, k, m * 128:(m + 1) * 128],
                                 h2[k][:], start=(k == 0), stop=(k == KT - 1))
            sg = dp.tile([128, TPC], F32, tag="gs", bufs=3, name="sg2")
            nc.scalar.activation(sg[:], pa[:], AF.Silu, bias=bcol(BG_S2A, m))
            o = dp.tile([128, TPC], F32, tag="out", bufs=2, name="o")
            nc.vector.scalar_tensor_tensor(o[:], pb[:], bcol(BG_S2B, m), sg[:],
                                           OP.add, OP.mult)
            nc.sync.dma_start(out_d[m * 128:(m + 1) * 128, :], o[:])

    # run_bass_via_pjrt serializes the BIR as-is; Bacc's lowering passes
    # (register allocation, TRN2 single-wait splitting) only run in
    # finalize(), so it must happen before dispatch.
    nc.finalize()
    return nc


def _pack8(w):
    """[1024, 1024] f32 -> [128, KT*1024] e4m3 rows, p-major contiguous."""
    q = np.clip(np.asarray(w, np.float32) * SW, -240, 240).astype(f8)
    return np.ascontiguousarray(
        q.reshape(KT, 128, 1024).transpose(1, 0, 2).reshape(128, KT * 1024))


def _pack16(w):
    q = np.asarray(w, np.float32).astype(bf16)
    return np.ascontiguousarray(
        q.reshape(KT, 128, 1024).transpose(1, 0, 2).reshape(128, KT * 1024))


def _pack_lin0n(w):
    """[1024, 1024] -> [128, KT*KT*128] e4m3, n-major then k then c."""
    q = np.clip(np.asarray(w, np.float32) * SW, -240, 240).astype(f8)
    # q[k*128+p, n*128+c] -> out[p, ((n*KT)+k)*128+c]
    q = q.reshape(KT, 128, KT, 128).transpose(1, 2, 0, 3)  # p, n, k, c
    return np.ascontiguousarray(q.reshape(128, KT * KT * 128))


def _pack_weights(inp):
    units = [_pack8(inp["swi1_w1"]), _pack8(inp["swi1_w2"])]
    w1, w3, w2 = (np.asarray(inp["exp_w1"], np.float32),
                  np.asarray(inp["exp_w3"], np.float32),
                  np.asarray(inp["exp_w2"], np.float32))
    for e in range(E):
        units += [_pack8(w1[e]), _pack8(w3[e]), _pack8(w2[e])]
    sh1 = np.asarray(inp["sh_w1"], np.float32)
    sh3 = np.asarray(inp["sh_w3"], np.float32)
    sh2 = np.asarray(inp["sh_w2"], np.float32)
    units += [_pack8(sh1[:, :1024]), _pack8(sh1[:, 1024:]),
              _pack8(sh3[:, :1024]), _pack8(sh3[:, 1024:]),
              _pack8(sh2[:1024, :]), _pack8(sh2[1024:, :])]
    assert len(units) == N_U8
    wall8 = np.ascontiguousarray(np.concatenate(units, axis=0))
    u16 = [_pack16(inp["lin1_w"]), _pack16(inp["swi2_w1"]), _pack16(inp["swi2_w2"])]
    wall16 = np.ascontiguousarray(np.concatenate(u16, axis=0))
    return wall8, wall16


def _pack_biases(inp):
    scaled = [
        ("lin0_b", SH0),
        ("swi1_b1", 1.0),
        ("swi1_b1", SH0 * SW),
        ("swi1_b2", SH0 * SW),
        ("lin1_b", 1.0),
        ("swi2_b1", 1.0),
        ("swi2_b2", 1.0),
    ]
    cols = []
    for name, s in scaled:
        v = (np.asarray(inp[name], np.float32) * s).reshape(KT, 128).T
        cols.append(v)
    return np.ascontiguousarray(np.concatenate(cols, axis=1))  # [128, 7*KT]


def kernel(**inputs):
    global _prog
    from concourse.bass_utils import run_bass_kernel_spmd

    if _prog is None:
        _prog = _build_program()
    nc = _prog

    wall8, wall16 = _pack_weights(inputs)
    lin0n = _pack_lin0n(inputs["lin0_w"])
    biases = _pack_biases(inputs)
    gq = np.clip(np.asarray(inputs["gate_w"], np.float32).T * SW, -240, 240).astype(f8)
    gq = np.ascontiguousarray(
        gq.reshape(KT, 128, E).transpose(1, 0, 2).reshape(128, KT * E))
    ident = np.eye(128, dtype=np.float32)
    sel = np.zeros((E, E * 128), dtype=np.float32)
    for e in range(E):
        sel[e, e * 128:(e + 1) * 128] = 1.0

    x = np.asarray(inputs["x"], np.float32).reshape(T, D)
    in_maps = []
    for c in range(NCORES):
        xT = np.clip(x[c * TPC:(c + 1) * TPC, :].T * SX, -240, 240).astype(f8)
        xqc = np.ascontiguousarray(
            xT.reshape(KT, 128, TPC).transpose(1, 0, 2).reshape(128, KT * TPC))
        in_maps.append({
            "wall8": wall8, "wall16": wall16, "lin0n": lin0n, "xq": xqc,
            "gq": gq, "biases": biases, "ident": ident, "sel": sel,
        })

    res = run_bass_kernel_spmd(nc, in_maps, list(range(NCORES)))
    global last_results
    last_results = res
    outT = np.concatenate([res.results[c]["outT"] for c in range(NCORES)], axis=1)
    return np.ascontiguousarray(outT.T).reshape(B, S, D).astype(np.float32)


# revision 18
# speedup vs baseline: 1.2653x; 1.0676x over previous
"""Trainium2 Bass kernel for the MoE feed-forward block (nn_MoEFF).

Strategy: pure data-parallel over tokens. The 4096 tokens are split into
8 slices of 512; each NeuronCore runs the *entire* network on its slice
(router + all 8 experts dense-masked + shared expert). With E=8/K=4 every
expert serves ~half the tokens anyway, so dense-masked expert compute
costs only 2x the ideal sparse flops and avoids all collectives.

Precision: the front block (lin0 + swiglu1), router, all routed experts
and the shared expert run in fp8 e4m3 with DoubleRow matmuls (2 fp8
weights per PE cell, 256-deep contraction per instruction -> ~1.76x
bf16 matmul throughput measured). Activations and weights carry static
power-of-2 scales chosen so sigma*scale ~ 8-20 (TRN e4m3 max normal is
240; every descale constant folds into an existing activation/vector
op). The back block (lin1 + swiglu2) stays bf16: fp8 noise there lands
directly on the output with no averaging downstream, and it is only
3/36 of the matmul work. Measured end-to-end rel err 7.1e-3.

Layout: activations are kept transposed in SBUF ([feature-on-partition,
token-on-free]) so weight matrices in natural [in, out] layout are the
stationary matmul operand. Weights are host-packed p-major so every
DMA row is contiguous; lin0 additionally ships n-major so its first
column chunk (and the first matmul) starts ~4us earlier. Each expert's
down-projection is emitted one expert late so its PSUM->fp8 vector
chain hides under the next expert's up-projection matmuls. All PSUM
accumulation is fp32.
"""

from contextlib import ExitStack

import ml_dtypes
import numpy as np

B, S, D = 2, 2048, 1024
E, TOPK, H = 8, 4, 1024
SH = 2 * H
NCORES = 8
T = B * S                 # 4096 tokens
TPC = T // NCORES         # 512 tokens per core
KT = D // 128             # 8 contraction tiles
KT2 = KT // 2             # 4 DoubleRow contraction steps
MT_H = H // 128           # 8
MT_SH = SH // 128         # 16
NT = TPC // 128           # 4 token sub-tiles (router)

bf16 = ml_dtypes.bfloat16
f8 = ml_dtypes.float8_e4m3  # TRN FP8_EXP4-compatible (max normal 240)

# static power-of-2 scales (data ~N(0, sigma) with known sigmas)
SW = 512.0                # fp8 weights (sigma 0.02 -> 10)
SM1 = 1024.0              # composed lin0@swi1_w matrices (sigma 0.0128)
SX = 16.0                 # x (sigma 1.0)
SHH = 64.0                # h, the MoE input (sigma 0.096, absmax 2.24)
SV = 4096.0               # expert/shared v = silu(g)*u (sigma 0.002)

# fp8 unit layout in wall8 (each unit is one [1024, 1024] matrix,
# host-packed to [128, KT*1024] p-major rows)
U_S1A = 0
U_S1B = 1
U_EXP = 2                 # 2 + 3*e + {0: w1, 1: w3, 2: w2}
U_SH1 = 26                # 26, 27: sh_w1 cols [0:1024], [1024:2048]
U_SH3 = 28                # 28, 29
U_SH2 = 30                # 30, 31: sh_w2 rows [0:1024], [1024:2048]
N_U8 = 32
# bf16 units in wall16 (composed lin1 @ swi2 weights)
U_M2A = 0
U_M2B = 1
N_U16 = 2

# bias-column groups in the const tensor [128, 4*KT]
BG_S1SG = 0               # composed b1a' = lin0_b@swi1_w1 + swi1_b1 (true)
BG_S1B = 1                # composed b1b' * SX*SM1
BG_S2A = 2                # composed b2a' = lin1_b@swi2_w1 + swi2_b1 (true)
BG_S2B = 3                # composed b2b' (true)

_prog = None  # built once per process
last_results = None  # BassKernelResults of the most recent kernel() call


def _build_program():
    import concourse.bacc as bacc
    import concourse.mybir as mybir
    import concourse.tile as tile

    F32, BF, F8 = mybir.dt.float32, mybir.dt.bfloat16, mybir.dt.float8e4
    AF = mybir.ActivationFunctionType
    OP = mybir.AluOpType
    DR = mybir.MatmulPerfMode.DoubleRow

    # descale constants (all exact powers of two)
    C_SG1 = 1.0 / (SX * SM1)          # sigmoid input descale, swiglu1
    C_H = SHH / (SX * SM1)            # f32 swiglu1 product -> fp8 h
    C_SGE = 1.0 / (SHH * SW)          # sigmoid input descale, experts/shared
    C_V = SV / (SHH * SW)             # expert/shared v -> fp8 (pre-combine)
    C_Y = 1.0 / (SV * SW)             # expert/shared y PSUM -> true-scale
    C_Z = 1.0 / (SHH * SW)            # router logits descale

    nc = bacc.Bacc()

    wall8_d = nc.dram_tensor("wall8", [N_U8 * 128, KT * 1024], F8,
                             kind="ExternalInput")
    wall16_d = nc.dram_tensor("wall16", [N_U16 * 128, KT * 1024], BF,
                              kind="ExternalInput")
    xq_d = nc.dram_tensor("xq", [128, KT * TPC], F8, kind="ExternalInput")
    gq_d = nc.dram_tensor("gq", [128, KT * E], F8, kind="ExternalInput")
    bias_d = nc.dram_tensor("biases", [128, 4 * KT], F32, kind="ExternalInput")
    ident_d = nc.dram_tensor("ident", [128, 128], F32, kind="ExternalInput")
    sel_d = nc.dram_tensor("sel", [E, E * 128], F32, kind="ExternalInput")
    out_d = nc.dram_tensor("outT", [D, TPC], F32, kind="ExternalOutput")

    with tile.TileContext(nc) as tc, ExitStack() as ctx:
        wp = ctx.enter_context(tc.tile_pool(name="wp", bufs=6))
        sp = ctx.enter_context(tc.tile_pool(name="sp", bufs=1))
        dp = ctx.enter_context(tc.tile_pool(name="dp", bufs=4))
        pp = ctx.enter_context(tc.tile_pool(name="pp", bufs=2, space="PSUM"))

        def wload8(unit):
            wt = wp.tile([128, KT, 1024], F8, tag="wmat8", bufs=7, name=f"w8_{unit}")
            nc.sync.dma_start(
                wt[:],
                wall8_d[unit * 128:(unit + 1) * 128, :].rearrange(
                    "p (k c) -> p k c", k=KT),
            )
            return wt

        def wload16(unit):
            wt = wp.tile([128, KT, 1024], BF, tag="wmat16", bufs=2, name=f"w16_{unit}")
            nc.sync.dma_start(
                wt[:],
                wall16_d[unit * 128:(unit + 1) * 128, :].rearrange(
                    "p (k c) -> p k c", k=KT),
            )
            return wt

        # ---- static inputs ----
        xq = sp.tile([128, KT, TPC], F8, tag="xq", name="xq")
        nc.sync.dma_start(xq[:], xq_d[:].rearrange("p (k t) -> p k t", k=KT))
        gq = sp.tile([128, KT, E], F8, tag="gq", name="gq")
        nc.sync.dma_start(gq[:], gq_d[:].rearrange("p (k e) -> p k e", k=KT))
        biases = sp.tile([128, 4 * KT], F32, tag="biases", name="biases")
        nc.sync.dma_start(biases[:], bias_d[:])
        ident = sp.tile([128, 128], F32, tag="ident", name="ident")
        nc.sync.dma_start(ident[:], ident_d[:])
        sel = sp.tile([E, E * 128], F32, tag="sel", name="sel")
        nc.sync.dma_start(sel[:], sel_d[:])

        def bcol(idx, n):
            # per-partition bias column n of bias group idx
            return biases[:, idx * KT + n:idx * KT + n + 1]

        def mm8(ps, wt, src, n):
            # DoubleRow accumulation: ps += wt[:, :, n-block].T @ src
            for q in range(KT2):
                nc.tensor.matmul(ps[:], wt[:, 2 * q:2 * q + 2, n * 128:(n + 1) * 128],
                                 src[:, 2 * q:2 * q + 2, :],
                                 start=(q == 0), stop=(q == KT2 - 1), perf_mode=DR)

        # ---- swiglu 1 -> h (the MoE input), fp8. lin0 is composed into
        # the swiglu weight matrices host-side, so x feeds straight in ----
        w_s1a, w_s1b = wload8(U_S1A), wload8(U_S1B)
        hq = sp.tile([128, KT, TPC], F8, tag="hq", name="hq")
        for m in range(KT):
            pa = pp.tile([128, TPC], F32, tag="gu", bufs=4, name="ps_a1")
            mm8(pa, w_s1a, xq, m)
            pb = pp.tile([128, TPC], F32, tag="gu", bufs=4, name="ps_b1")
            mm8(pb, w_s1b, xq, m)
            sg = dp.tile([128, TPC], F32, tag="gs", bufs=3, name="sg1")
            nc.scalar.activation(sg[:], pa[:], AF.Silu,
                                 bias=bcol(BG_S1SG, m), scale=C_SG1)
            t = dp.tile([128, TPC], F32, tag="v", bufs=3, name="t1")
            nc.vector.scalar_tensor_tensor(t[:], pb[:], bcol(BG_S1B, m), sg[:],
                                           OP.add, OP.mult)
            nc.scalar.activation(hq[:, m, :], t[:], AF.Copy, scale=C_H)

        # ---- router matmuls: z[t, e] for all 4 token sub-tiles ----
        z_all = pp.tile([128, NT * E], F32, tag="misc", bufs=2, name="z_all")
        for t in range(NT):
            for k in range(KT):
                nc.tensor.matmul(z_all[:, t * E:(t + 1) * E],
                                 hq[:, k, t * 128:(t + 1) * 128],
                                 gq[:, k, :], start=(k == 0), stop=(k == KT - 1))

        # ---- router chain (DVE/ACT; overlaps expert-0 g/u matmuls on PE) ----
        ez = sp.tile([128, NT * E], F32, tag="ez", name="ez")
        cur = sp.tile([128, NT * E], F32, tag="cur", name="cur")
        cm = sp.tile([128, NT * E], F32, tag="cm", name="cm")
        combine = sp.tile([128, NT * E], F32, tag="combine", name="combine")
        stat = sp.tile([128, 4 * NT], F32, tag="stat", name="stat")  # nmx, thr, s, r

        for t in range(NT):
            zt = z_all[:, t * E:(t + 1) * E]
            nmx = stat[:, t:t + 1]
            nc.vector.tensor_reduce(nmx, zt, mybir.AxisListType.X, OP.max, negate=True)
            # nmx holds -max of the scaled logits; Exp gets scale applied to
            # the input only, so pre-scale the bias to true units
            nc.vector.tensor_scalar(nmx, nmx, C_Z, None, OP.mult)
            ezt = ez[:, t * E:(t + 1) * E]
            nc.scalar.activation(ezt, zt, AF.Exp, bias=nmx, scale=C_Z)
            curt = cur[:, t * E:(t + 1) * E]
            nc.vector.tensor_copy(curt, ezt)
            thr = stat[:, NT + t:NT + t + 1]
            for i in range(TOPK):
                nc.vector.tensor_reduce(thr, curt, mybir.AxisListType.X, OP.max)
                if i < TOPK - 1:
                    eq = dp.tile([128, E], F32, tag="eq", bufs=2, name="eq")
                    nc.vector.tensor_scalar(eq[:], curt, thr, None, OP.is_equal)
                    nc.vector.scalar_tensor_tensor(curt, eq[:], -1e30, curt,
                                                   OP.mult, OP.add)
            cmt = cm[:, t * E:(t + 1) * E]
            # cm = ez * (ez >= thr); reuse cur as the mask buffer
            nc.vector.tensor_scalar(curt, ezt, thr, None, OP.is_ge)
            nc.vector.tensor_mul(cmt, ezt, curt)
            s = stat[:, 2 * NT + t:2 * NT + t + 1]
            nc.vector.tensor_reduce(s, cmt, mybir.AxisListType.X, OP.add)
            r = stat[:, 3 * NT + t:3 * NT + t + 1]
            nc.vector.reciprocal(r, s)
            nc.vector.tensor_scalar(combine[:, t * E:(t + 1) * E], cmt, r, None,
                                    OP.mult)

        cbT = sp.tile([E, TPC], F32, tag="cbT", name="cbT")

        def emit_transposes():
            for t in range(NT):
                trp = pp.tile([E, 128], F32, tag="misc", name="trp")
                nc.tensor.transpose(trp[:], combine[:, t * E:(t + 1) * E], ident[:])
                nc.scalar.activation(cbT[0:E, t * 128:(t + 1) * 128], trp[:], AF.Copy)

        def outer(e):
            # cb_ps[p, t] = sum_k sel[k, e*128+p] * cbT[k, t] = combine[t, e]
            cb_ps = pp.tile([128, TPC], F32, tag="misc", name="cb_ps")
            nc.tensor.matmul(cb_ps[:], sel[:, e * 128:(e + 1) * 128], cbT[0:E, :],
                             start=True, stop=True)
            return cb_ps

        def emit_gu(w1, w3, n_m, cb_ps, tag):
            """g/u/v for one expert (n_m m-tiles) -> fp8 vb tile
            [128, n_m, TPC], scaled by cb_ps when given."""
            vb = dp.tile([128, n_m, TPC], F8, tag=tag, bufs=(3 if n_m == MT_H else 1), name="vb")
            for m in range(n_m):
                u, mm = divmod(m, KT)
                pg = pp.tile([128, TPC], F32, tag="gu", bufs=4, name="ps_g")
                mm8(pg, w1[u], hq, mm)
                pu = pp.tile([128, TPC], F32, tag="gu", bufs=4, name="ps_u")
                mm8(pu, w3[u], hq, mm)
                sg = dp.tile([128, TPC], F32, tag="gs", bufs=3, name="sg")
                nc.scalar.activation(sg[:], pg[:], AF.Silu, scale=C_SGE)
                if cb_ps is None:
                    nc.vector.scalar_tensor_tensor(vb[:, m, :], pu[:], C_V, sg[:],
                                                   OP.mult, OP.mult)
                else:
                    v = dp.tile([128, TPC], F32, tag="v", bufs=3, name="v")
                    nc.vector.scalar_tensor_tensor(v[:], pu[:], C_V, sg[:],
                                                   OP.mult, OP.mult)
                    nc.vector.tensor_mul(vb[:, m, :], v[:], cb_ps[:])
            return vb

        def emit_y(w2, vb, n_m, acc, cb_sb=None, final=None):
            """y = vb @ w2 accumulated into acc (fp32 SBUF). w2: list of units.
            final: list to receive bf16 direct-out tiles (last stage)."""
            for n in range(KT):
                py = pp.tile([128, TPC], F32, tag="y", bufs=2, name="ps_y")
                for q in range(n_m // 2):
                    u, qq = divmod(q, KT2)
                    nc.tensor.matmul(py[:],
                                     w2[u][:, 2 * qq:2 * qq + 2, n * 128:(n + 1) * 128],
                                     vb[:, 2 * q:2 * q + 2, :],
                                     start=(q == 0), stop=(q == n_m // 2 - 1),
                                     perf_mode=DR)
                if cb_sb is not None:
                    a = sp.tile([128, TPC], F32, tag="acc", bufs=8, name=f"acc_{n}")
                    nc.vector.tensor_mul(a[:], py[:], cb_sb[:])
                    acc.append(a)
                elif final is not None:
                    t = sp.tile([128, TPC], BF, tag="accbf", bufs=8,
                                name=f"accbf_{n}")
                    nc.vector.scalar_tensor_tensor(t[:], py[:], C_Y, acc[n][:],
                                                   OP.mult, OP.add)
                    final.append(t)
                else:
                    nc.vector.scalar_tensor_tensor(acc[n][:], py[:], C_Y, acc[n][:],
                                                   OP.mult, OP.add)

        # ---- experts (fp8; expert 0 applies combine on the output side,
        # the rest fold it into vb before the w2 matmul) ----
        acc = []
        vb_prev = w2_prev = cb_sb0 = None
        for e in range(E):
            we1 = wload8(U_EXP + 3 * e)
            we3 = wload8(U_EXP + 3 * e + 1)
            we2 = wload8(U_EXP + 3 * e + 2)
            if e == 0:
                vb = emit_gu([we1], [we3], MT_H, None, "vb")
                emit_transposes()
                cb_ps0 = outer(0)
                cb_sb0 = dp.tile([128, TPC], F32, tag="cbsb", bufs=1, name="cb_sb0")
                # descale C_Y folded in: acc = py * (combine * C_Y)
                nc.scalar.activation(cb_sb0[:], cb_ps0[:], AF.Copy, scale=C_Y)
            else:
                cb_ps = outer(e)
                vb = emit_gu([we1], [we3], MT_H, cb_ps, "vb")
                emit_y([w2_prev], vb_prev, MT_H, acc,
                       cb_sb=(cb_sb0 if e == 1 else None))
            vb_prev, w2_prev = vb, we2

        # ---- shared expert (always-on, unscaled) ----
        sh1 = [wload8(U_SH1), wload8(U_SH1 + 1)]
        sh3 = [wload8(U_SH3), wload8(U_SH3 + 1)]
        sh2 = [wload8(U_SH2), wload8(U_SH2 + 1)]
        emit_y([w2_prev], vb_prev, MT_H, acc)
        vbsh = emit_gu(sh1, sh3, MT_SH, None, "vbsh")
        w_s2a = wload16(U_M2A)
        w_s2b = wload16(U_M2B)
        accbf = []
        emit_y(sh2, vbsh, MT_SH, acc, final=accbf)

        # ---- block 3: swiglu2 with lin1 composed in (bf16, true scale) ----
        for m in range(KT):
            pa = pp.tile([128, TPC], F32, tag="gu", bufs=4, name="ps_a2")
            for k in range(KT):
                nc.tensor.matmul(pa[:], w_s2a[:, k, m * 128:(m + 1) * 128],
                                 accbf[k][:], start=(k == 0), stop=(k == KT - 1))
            pb = pp.tile([128, TPC], F32, tag="gu", bufs=4, name="ps_b2")
            for k in range(KT):
                nc.tensor.matmul(pb[:], w_s2b[:, k, m * 128:(m + 1) * 128],
                                 accbf[k][:], start=(k == 0), stop=(k == KT - 1))
            sg = dp.tile([128, TPC], F32, tag="gs", bufs=3, name="sg2")
            nc.scalar.activation(sg[:], pa[:], AF.Silu, bias=bcol(BG_S2A, m))
            o = dp.tile([128, TPC], F32, tag="out", bufs=2, name="o")
            nc.vector.scalar_tensor_tensor(o[:], pb[:], bcol(BG_S2B, m), sg[:],
                                           OP.add, OP.mult)
            nc.sync.dma_start(out_d[m * 128:(m + 1) * 128, :], o[:])

    # run_bass_via_pjrt serializes the BIR as-is; Bacc's lowering passes
    # (register allocation, TRN2 single-wait splitting) only run in
    # finalize(), so it must happen before dispatch.
    nc.finalize()
    return nc


def _pack8(w, s=SW):
    """[1024, 1024] f32 -> [128, KT*1024] e4m3 rows, p-major contiguous."""
    q = np.clip(np.asarray(w, np.float32) * s, -240, 240).astype(f8)
    return np.ascontiguousarray(
        q.reshape(KT, 128, 1024).transpose(1, 0, 2).reshape(128, KT * 1024))


def _pack16(w):
    q = np.asarray(w, np.float32).astype(bf16)
    return np.ascontiguousarray(
        q.reshape(KT, 128, 1024).transpose(1, 0, 2).reshape(128, KT * 1024))


def _compose(inp):
    """Fold the two pure-linear layers into their following swiglu weights:
    (x@W0+b0)@W1 + b1 == x@(W0@W1) + (b0@W1 + b1). Exact linear algebra;
    it removes two matmul stages and two quantization round-trips."""
    W0 = np.asarray(inp["lin0_w"], np.float64)
    b0 = np.asarray(inp["lin0_b"], np.float64)
    W1 = np.asarray(inp["lin1_w"], np.float64)
    b1 = np.asarray(inp["lin1_b"], np.float64)
    M1a = W0 @ np.asarray(inp["swi1_w1"], np.float64)
    M1b = W0 @ np.asarray(inp["swi1_w2"], np.float64)
    b1a = b0 @ np.asarray(inp["swi1_w1"], np.float64) + inp["swi1_b1"]
    b1b = b0 @ np.asarray(inp["swi1_w2"], np.float64) + inp["swi1_b2"]
    M2a = W1 @ np.asarray(inp["swi2_w1"], np.float64)
    M2b = W1 @ np.asarray(inp["swi2_w2"], np.float64)
    b2a = b1 @ np.asarray(inp["swi2_w1"], np.float64) + inp["swi2_b1"]
    b2b = b1 @ np.asarray(inp["swi2_w2"], np.float64) + inp["swi2_b2"]
    return (M1a, M1b, M2a, M2b,
            b1a.astype(np.float32), b1b.astype(np.float32),
            b2a.astype(np.float32), b2b.astype(np.float32))


def _pack_weights(inp, M1a, M1b, M2a, M2b):
    units = [_pack8(M1a, SM1), _pack8(M1b, SM1)]
    w1, w3, w2 = (np.asarray(inp["exp_w1"], np.float32),
                  np.asarray(inp["exp_w3"], np.float32),
                  np.asarray(inp["exp_w2"], np.float32))
    for e in range(E):
        units += [_pack8(w1[e]), _pack8(w3[e]), _pack8(w2[e])]
    sh1 = np.asarray(inp["sh_w1"], np.float32)
    sh3 = np.asarray(inp["sh_w3"], np.float32)
    sh2 = np.asarray(inp["sh_w2"], np.float32)
    units += [_pack8(sh1[:, :1024]), _pack8(sh1[:, 1024:]),
              _pack8(sh3[:, :1024]), _pack8(sh3[:, 1024:]),
              _pack8(sh2[:1024, :]), _pack8(sh2[1024:, :])]
    assert len(units) == N_U8
    wall8 = np.ascontiguousarray(np.concatenate(units, axis=0))
    u16 = [_pack16(M2a), _pack16(M2b)]
    wall16 = np.ascontiguousarray(np.concatenate(u16, axis=0))
    return wall8, wall16


def _pack_biases(b1a, b1b, b2a, b2b):
    cols = []
    for v, s in [(b1a, 1.0), (b1b, SX * SM1), (b2a, 1.0), (b2b, 1.0)]:
        cols.append((v * s).reshape(KT, 128).T.astype(np.float32))
    return np.ascontiguousarray(np.concatenate(cols, axis=1))  # [128, 4*KT]


def kernel(**inputs):
    global _prog
    from concourse.bass_utils import run_bass_kernel_spmd

    if _prog is None:
        _prog = _build_program()
    nc = _prog

    M1a, M1b, M2a, M2b, b1a, b1b, b2a, b2b = _compose(inputs)
    wall8, wall16 = _pack_weights(inputs, M1a, M1b, M2a, M2b)
    biases = _pack_biases(b1a, b1b, b2a, b2b)
    gq = np.clip(np.asarray(inputs["gate_w"], np.float32).T * SW, -240, 240).astype(f8)
    gq = np.ascontiguousarray(
        gq.reshape(KT, 128, E).transpose(1, 0, 2).reshape(128, KT * E))
    ident = np.eye(128, dtype=np.float32)
    sel = np.zeros((E, E * 128), dtype=np.float32)
    for e in range(E):
        sel[e, e * 128:(e + 1) * 128] = 1.0

    x = np.asarray(inputs["x"], np.float32).reshape(T, D)
    in_maps = []
    for c in range(NCORES):
        xT = np.clip(x[c * TPC:(c + 1) * TPC, :].T * SX, -240, 240).astype(f8)
        xqc = np.ascontiguousarray(
            xT.reshape(KT, 128, TPC).transpose(1, 0, 2).reshape(128, KT * TPC))
        in_maps.append({
            "wall8": wall8, "wall16": wall16, "xq": xqc,
            "gq": gq, "biases": biases, "ident": ident, "sel": sel,
        })

    res = run_bass_kernel_spmd(nc, in_maps, list(range(NCORES)))
    global last_results
    last_results = res
    outT = np.concatenate([res.results[c]["outT"] for c in range(NCORES)], axis=1)
    return np.ascontiguousarray(outT.T).reshape(B, S, D).astype(np.float32)


# revision 19
# speedup vs baseline: 1.2863x; 1.0166x over previous
"""Trainium2 Bass kernel for the MoE feed-forward block (nn_MoEFF).

Strategy: pure data-parallel over tokens. The 4096 tokens are split into
8 slices of 512; each NeuronCore runs the *entire* network on its slice
(router + all 8 experts dense-masked + shared expert). With E=8/K=4 every
expert serves ~half the tokens anyway, so dense-masked expert compute
costs only 2x the ideal sparse flops and avoids all collectives.

Precision: the front block (lin0 + swiglu1), router, all routed experts
and the shared expert run in fp8 e4m3 with DoubleRow matmuls (2 fp8
weights per PE cell, 256-deep contraction per instruction -> ~1.76x
bf16 matmul throughput measured). Activations and weights carry static
power-of-2 scales chosen so sigma*scale ~ 8-20 (TRN e4m3 max normal is
240; every descale constant folds into an existing activation/vector
op). The back block (lin1 + swiglu2) stays bf16: fp8 noise there lands
directly on the output with no averaging downstream, and it is only
3/36 of the matmul work. Measured end-to-end rel err 7.1e-3.

Layout: activations are kept transposed in SBUF ([feature-on-partition,
token-on-free]) so weight matrices in natural [in, out] layout are the
stationary matmul operand. Weights are host-packed p-major so every
DMA row is contiguous; lin0 additionally ships n-major so its first
column chunk (and the first matmul) starts ~4us earlier. Each expert's
down-projection is emitted one expert late so its PSUM->fp8 vector
chain hides under the next expert's up-projection matmuls. All PSUM
accumulation is fp32.
"""

from contextlib import ExitStack

import ml_dtypes
import numpy as np

B, S, D = 2, 2048, 1024
E, TOPK, H = 8, 4, 1024
SH = 2 * H
NCORES = 8
T = B * S                 # 4096 tokens
TPC = T // NCORES         # 512 tokens per core
KT = D // 128             # 8 contraction tiles
KT2 = KT // 2             # 4 DoubleRow contraction steps
MT_H = H // 128           # 8
MT_SH = SH // 128         # 16
NT = TPC // 128           # 4 token sub-tiles (router)

bf16 = ml_dtypes.bfloat16
f8 = ml_dtypes.float8_e4m3  # TRN FP8_EXP4-compatible (max normal 240)

# static power-of-2 scales (data ~N(0, sigma) with known sigmas)
SW = 512.0                # fp8 weights (sigma 0.02 -> 10)
SM1 = 1024.0              # composed lin0@swi1_w matrices (sigma 0.0128)
SX = 16.0                 # x (sigma 1.0)
SHH = 64.0                # h, the MoE input (sigma 0.096, absmax 2.24)
SV = 4096.0               # expert/shared v = silu(g)*u (sigma 0.002)

# fp8 unit layout in wall8 (each unit is one [1024, 1024] matrix,
# host-packed to [128, KT*1024] p-major rows). The composed swiglu1
# matrices ship separately (n-major) for chunked early-start loads.
U_EXP = 0                 # 3*e + {0: w1, 1: w3, 2: w2}
U_SH1 = 24                # 24, 25: sh_w1 cols [0:1024], [1024:2048]
U_SH3 = 26                # 26, 27
U_SH2 = 28                # 28, 29: sh_w2 rows [0:1024], [1024:2048]
N_U8 = 30
# bf16 units in wall16 (composed lin1 @ swi2 weights)
U_M2A = 0
U_M2B = 1
N_U16 = 2

# bias-column groups in the const tensor [128, 4*KT]
BG_S1SG = 0               # composed b1a' = lin0_b@swi1_w1 + swi1_b1 (true)
BG_S1B = 1                # composed b1b' * SX*SM1
BG_S2A = 2                # composed b2a' = lin1_b@swi2_w1 + swi2_b1 (true)
BG_S2B = 3                # composed b2b' (true)

_prog = None  # built once per process
last_results = None  # BassKernelResults of the most recent kernel() call


def _build_program():
    import concourse.bacc as bacc
    import concourse.mybir as mybir
    import concourse.tile as tile

    F32, BF, F8 = mybir.dt.float32, mybir.dt.bfloat16, mybir.dt.float8e4
    AF = mybir.ActivationFunctionType
    OP = mybir.AluOpType
    DR = mybir.MatmulPerfMode.DoubleRow

    # descale constants (all exact powers of two)
    C_SG1 = 1.0 / (SX * SM1)          # sigmoid input descale, swiglu1
    C_H = SHH / (SX * SM1)            # f32 swiglu1 product -> fp8 h
    C_SGE = 1.0 / (SHH * SW)          # sigmoid input descale, experts/shared
    C_V = SV / (SHH * SW)             # expert/shared v -> fp8 (pre-combine)
    C_Y = 1.0 / (SV * SW)             # expert/shared y PSUM -> true-scale
    C_Z = 1.0 / (SHH * SW)            # router logits descale

    nc = bacc.Bacc()

    wall8_d = nc.dram_tensor("wall8", [N_U8 * 128, KT * 1024], F8,
                             kind="ExternalInput")
    wall16_d = nc.dram_tensor("wall16", [N_U16 * 128, KT * 1024], BF,
                              kind="ExternalInput")
    m1an_d = nc.dram_tensor("m1an", [128, KT * 1024], F8, kind="ExternalInput")
    m1bn_d = nc.dram_tensor("m1bn", [128, KT * 1024], F8, kind="ExternalInput")
    xq_d = nc.dram_tensor("xq", [128, KT * TPC], F8, kind="ExternalInput")
    gq_d = nc.dram_tensor("gq", [128, KT * E], F8, kind="ExternalInput")
    bias_d = nc.dram_tensor("biases", [128, 4 * KT], F32, kind="ExternalInput")
    ident_d = nc.dram_tensor("ident", [128, 128], F32, kind="ExternalInput")
    sel_d = nc.dram_tensor("sel", [E, E * 128], F32, kind="ExternalInput")
    out_d = nc.dram_tensor("outT", [D, TPC], F32, kind="ExternalOutput")

    with tile.TileContext(nc) as tc, ExitStack() as ctx:
        wp = ctx.enter_context(tc.tile_pool(name="wp", bufs=6))
        sp = ctx.enter_context(tc.tile_pool(name="sp", bufs=1))
        dp = ctx.enter_context(tc.tile_pool(name="dp", bufs=4))
        pp = ctx.enter_context(tc.tile_pool(name="pp", bufs=2, space="PSUM"))

        def wload8(unit):
            wt = wp.tile([128, KT, 1024], F8, tag="wmat8", bufs=7, name=f"w8_{unit}")
            nc.sync.dma_start(
                wt[:],
                wall8_d[unit * 128:(unit + 1) * 128, :].rearrange(
                    "p (k c) -> p k c", k=KT),
            )
            return wt

        def wload16(unit):
            wt = wp.tile([128, KT, 1024], BF, tag="wmat16", bufs=2, name=f"w16_{unit}")
            nc.sync.dma_start(
                wt[:],
                wall16_d[unit * 128:(unit + 1) * 128, :].rearrange(
                    "p (k c) -> p k c", k=KT),
            )
            return wt

        # ---- static inputs ----
        xq = sp.tile([128, KT, TPC], F8, tag="xq", name="xq")
        nc.sync.dma_start(xq[:], xq_d[:].rearrange("p (k t) -> p k t", k=KT))
        gq = sp.tile([128, KT, E], F8, tag="gq", name="gq")
        nc.sync.dma_start(gq[:], gq_d[:].rearrange("p (k e) -> p k e", k=KT))
        biases = sp.tile([128, 4 * KT], F32, tag="biases", name="biases")
        nc.sync.dma_start(biases[:], bias_d[:])
        ident = sp.tile([128, 128], F32, tag="ident", name="ident")
        nc.sync.dma_start(ident[:], ident_d[:])
        sel = sp.tile([E, E * 128], F32, tag="sel", name="sel")
        nc.sync.dma_start(sel[:], sel_d[:])

        def bcol(idx, n):
            # per-partition bias column n of bias group idx
            return biases[:, idx * KT + n:idx * KT + n + 1]

        def mm8(ps, wt, src, n):
            # DoubleRow accumulation: ps += wt[:, :, n-block].T @ src
            for q in range(KT2):
                nc.tensor.matmul(ps[:], wt[:, 2 * q:2 * q + 2, n * 128:(n + 1) * 128],
                                 src[:, 2 * q:2 * q + 2, :],
                                 start=(q == 0), stop=(q == KT2 - 1), perf_mode=DR)

        # ---- swiglu 1 -> h (the MoE input), fp8. lin0 is composed into
        # the swiglu weight matrices host-side, so x feeds straight in.
        # Both matrices arrive as 8 contiguous column chunks (n-major host
        # pack) so the m=0 matmuls start after 2x128KB instead of 2x1MB ----
        w_s1a = wp.tile([128, KT, KT, 128], F8, tag="wmat8", bufs=7, name="w_m1a")
        w_s1b = wp.tile([128, KT, KT, 128], F8, tag="wmat8", bufs=7, name="w_m1b")
        m1a_src = m1an_d[:].rearrange("p (n k c) -> p n k c", n=KT, k=KT)
        m1b_src = m1bn_d[:].rearrange("p (n k c) -> p n k c", n=KT, k=KT)
        for n in range(KT):
            nc.sync.dma_start(w_s1a[:, n, :, :], m1a_src[:, n, :, :])
            nc.sync.dma_start(w_s1b[:, n, :, :], m1b_src[:, n, :, :])
        hq = sp.tile([128, KT, TPC], F8, tag="hq", name="hq")
        for m in range(KT):
            pa = pp.tile([128, TPC], F32, tag="gu", bufs=4, name="ps_a1")
            for q in range(KT2):
                nc.tensor.matmul(pa[:], w_s1a[:, m, 2 * q:2 * q + 2, :],
                                 xq[:, 2 * q:2 * q + 2, :],
                                 start=(q == 0), stop=(q == KT2 - 1), perf_mode=DR)
            pb = pp.tile([128, TPC], F32, tag="gu", bufs=4, name="ps_b1")
            for q in range(KT2):
                nc.tensor.matmul(pb[:], w_s1b[:, m, 2 * q:2 * q + 2, :],
                                 xq[:, 2 * q:2 * q + 2, :],
                                 start=(q == 0), stop=(q == KT2 - 1), perf_mode=DR)
            sg = dp.tile([128, TPC], F32, tag="gs", bufs=3, name="sg1")
            nc.scalar.activation(sg[:], pa[:], AF.Silu,
                                 bias=bcol(BG_S1SG, m), scale=C_SG1)
            t = dp.tile([128, TPC], F32, tag="v", bufs=3, name="t1")
            nc.vector.scalar_tensor_tensor(t[:], pb[:], bcol(BG_S1B, m), sg[:],
                                           OP.add, OP.mult)
            nc.scalar.activation(hq[:, m, :], t[:], AF.Copy, scale=C_H)

        # ---- router matmuls: z[t, e] for all 4 token sub-tiles ----
        z_all = pp.tile([128, NT * E], F32, tag="misc", bufs=2, name="z_all")
        for t in range(NT):
            for k in range(KT):
                nc.tensor.matmul(z_all[:, t * E:(t + 1) * E],
                                 hq[:, k, t * 128:(t + 1) * 128],
                                 gq[:, k, :], start=(k == 0), stop=(k == KT - 1))

        # ---- router chain (DVE/ACT; overlaps expert-0 g/u matmuls on PE) ----
        ez = sp.tile([128, NT * E], F32, tag="ez", name="ez")
        cur = sp.tile([128, NT * E], F32, tag="cur", name="cur")
        cm = sp.tile([128, NT * E], F32, tag="cm", name="cm")
        combine = sp.tile([128, NT * E], F32, tag="combine", name="combine")
        stat = sp.tile([128, 4 * NT], F32, tag="stat", name="stat")  # nmx, thr, s, r

        for t in range(NT):
            zt = z_all[:, t * E:(t + 1) * E]
            nmx = stat[:, t:t + 1]
            nc.vector.tensor_reduce(nmx, zt, mybir.AxisListType.X, OP.max, negate=True)
            # nmx holds -max of the scaled logits; Exp gets scale applied to
            # the input only, so pre-scale the bias to true units
            nc.vector.tensor_scalar(nmx, nmx, C_Z, None, OP.mult)
            ezt = ez[:, t * E:(t + 1) * E]
            nc.scalar.activation(ezt, zt, AF.Exp, bias=nmx, scale=C_Z)
            curt = cur[:, t * E:(t + 1) * E]
            nc.vector.tensor_copy(curt, ezt)
            thr = stat[:, NT + t:NT + t + 1]
            for i in range(TOPK):
                nc.vector.tensor_reduce(thr, curt, mybir.AxisListType.X, OP.max)
                if i < TOPK - 1:
                    eq = dp.tile([128, E], F32, tag="eq", bufs=2, name="eq")
                    nc.vector.tensor_scalar(eq[:], curt, thr, None, OP.is_equal)
                    nc.vector.scalar_tensor_tensor(curt, eq[:], -1e30, curt,
                                                   OP.mult, OP.add)
            cmt = cm[:, t * E:(t + 1) * E]
            # cm = ez * (ez >= thr); reuse cur as the mask buffer
            nc.vector.tensor_scalar(curt, ezt, thr, None, OP.is_ge)
            nc.vector.tensor_mul(cmt, ezt, curt)
            s = stat[:, 2 * NT + t:2 * NT + t + 1]
            nc.vector.tensor_reduce(s, cmt, mybir.AxisListType.X, OP.add)
            r = stat[:, 3 * NT + t:3 * NT + t + 1]
            nc.vector.reciprocal(r, s)
            nc.vector.tensor_scalar(combine[:, t * E:(t + 1) * E], cmt, r, None,
                                    OP.mult)

        cbT = sp.tile([E, TPC], F32, tag="cbT", name="cbT")

        def emit_transposes():
            for t in range(NT):
                trp = pp.tile([E, 128], F32, tag="misc", name="trp")
                nc.tensor.transpose(trp[:], combine[:, t * E:(t + 1) * E], ident[:])
                nc.scalar.activation(cbT[0:E, t * 128:(t + 1) * 128], trp[:], AF.Copy)

        def outer(e):
            # cb_ps[p, t] = sum_k sel[k, e*128+p] * cbT[k, t] = combine[t, e]
            cb_ps = pp.tile([128, TPC], F32, tag="misc", name="cb_ps")
            nc.tensor.matmul(cb_ps[:], sel[:, e * 128:(e + 1) * 128], cbT[0:E, :],
                             start=True, stop=True)
            return cb_ps

        def emit_gu(w1, w3, n_m, cb_ps, tag):
            """g/u/v for one expert (n_m m-tiles) -> fp8 vb tile
            [128, n_m, TPC], scaled by cb_ps when given."""
            vb = dp.tile([128, n_m, TPC], F8, tag=tag, bufs=(3 if n_m == MT_H else 1), name="vb")
            for m in range(n_m):
                u, mm = divmod(m, KT)
                pg = pp.tile([128, TPC], F32, tag="gu", bufs=4, name="ps_g")
                mm8(pg, w1[u], hq, mm)
                pu = pp.tile([128, TPC], F32, tag="gu", bufs=4, name="ps_u")
                mm8(pu, w3[u], hq, mm)
                sg = dp.tile([128, TPC], F32, tag="gs", bufs=3, name="sg")
                nc.scalar.activation(sg[:], pg[:], AF.Silu, scale=C_SGE)
                if cb_ps is None:
                    nc.vector.scalar_tensor_tensor(vb[:, m, :], pu[:], C_V, sg[:],
                                                   OP.mult, OP.mult)
                else:
                    v = dp.tile([128, TPC], F32, tag="v", bufs=3, name="v")
                    nc.vector.scalar_tensor_tensor(v[:], pu[:], C_V, sg[:],
                                                   OP.mult, OP.mult)
                    nc.vector.tensor_mul(vb[:, m, :], v[:], cb_ps[:])
            return vb

        def emit_y(w2, vb, n_m, acc, cb_sb=None, final=None):
            """y = vb @ w2 accumulated into acc (fp32 SBUF). w2: list of units.
            final: list to receive bf16 direct-out tiles (last stage)."""
            for n in range(KT):
                py = pp.tile([128, TPC], F32, tag="y", bufs=2, name="ps_y")
                for q in range(n_m // 2):
                    u, qq = divmod(q, KT2)
                    nc.tensor.matmul(py[:],
                                     w2[u][:, 2 * qq:2 * qq + 2, n * 128:(n + 1) * 128],
                                     vb[:, 2 * q:2 * q + 2, :],
                                     start=(q == 0), stop=(q == n_m // 2 - 1),
                                     perf_mode=DR)
                if cb_sb is not None:
                    a = sp.tile([128, TPC], F32, tag="acc", bufs=8, name=f"acc_{n}")
                    nc.vector.tensor_mul(a[:], py[:], cb_sb[:])
                    acc.append(a)
                elif final is not None:
                    t = sp.tile([128, TPC], BF, tag="accbf", bufs=8,
                                name=f"accbf_{n}")
                    nc.vector.scalar_tensor_tensor(t[:], py[:], C_Y, acc[n][:],
                                                   OP.mult, OP.add)
                    final.append(t)
                else:
                    nc.vector.scalar_tensor_tensor(acc[n][:], py[:], C_Y, acc[n][:],
                                                   OP.mult, OP.add)

        # ---- experts (fp8; expert 0 applies combine on the output side,
        # the rest fold it into vb before the w2 matmul) ----
        acc = []
        vb_prev = w2_prev = cb_sb0 = None
        for e in range(E):
            we1 = wload8(U_EXP + 3 * e)
            we3 = wload8(U_EXP + 3 * e + 1)
            we2 = wload8(U_EXP + 3 * e + 2)
            if e == 0:
                vb = emit_gu([we1], [we3], MT_H, None, "vb")
                emit_transposes()
                cb_ps0 = outer(0)
                cb_sb0 = dp.tile([128, TPC], F32, tag="cbsb", bufs=1, name="cb_sb0")
                # descale C_Y folded in: acc = py * (combine * C_Y)
                nc.scalar.activation(cb_sb0[:], cb_ps0[:], AF.Copy, scale=C_Y)
            else:
                cb_ps = outer(e)
                vb = emit_gu([we1], [we3], MT_H, cb_ps, "vb")
                emit_y([w2_prev], vb_prev, MT_H, acc,
                       cb_sb=(cb_sb0 if e == 1 else None))
            vb_prev, w2_prev = vb, we2

        # ---- shared expert (always-on, unscaled) ----
        sh1 = [wload8(U_SH1), wload8(U_SH1 + 1)]
        sh3 = [wload8(U_SH3), wload8(U_SH3 + 1)]
        sh2 = [wload8(U_SH2), wload8(U_SH2 + 1)]
        emit_y([w2_prev], vb_prev, MT_H, acc)
        vbsh = emit_gu(sh1, sh3, MT_SH, None, "vbsh")
        w_s2a = wload16(U_M2A)
        w_s2b = wload16(U_M2B)
        accbf = []
        emit_y(sh2, vbsh, MT_SH, acc, final=accbf)

        # ---- block 3: swiglu2 with lin1 composed in (bf16, true scale) ----
        for m in range(KT):
            pa = pp.tile([128, TPC], F32, tag="gu", bufs=4, name="ps_a2")
            for k in range(KT):
                nc.tensor.matmul(pa[:], w_s2a[:, k, m * 128:(m + 1) * 128],
                                 accbf[k][:], start=(k == 0), stop=(k == KT - 1))
            pb = pp.tile([128, TPC], F32, tag="gu", bufs=4, name="ps_b2")
            for k in range(KT):
                nc.tensor.matmul(pb[:], w_s2b[:, k, m * 128:(m + 1) * 128],
                                 accbf[k][:], start=(k == 0), stop=(k == KT - 1))
            sg = dp.tile([128, TPC], F32, tag="gs", bufs=3, name="sg2")
            nc.scalar.activation(sg[:], pa[:], AF.Silu, bias=bcol(BG_S2A, m))
            o = dp.tile([128, TPC], F32, tag="out", bufs=2, name="o")
            nc.vector.scalar_tensor_tensor(o[:], pb[:], bcol(BG_S2B, m), sg[:],
                                           OP.add, OP.mult)
            nc.sync.dma_start(out_d[m * 128:(m + 1) * 128, :], o[:])

    # run_bass_via_pjrt serializes the BIR as-is; Bacc's lowering passes
    # (register allocation, TRN2 single-wait splitting) only run in
    # finalize(), so it must happen before dispatch.
    nc.finalize()
    return nc


def _pack8(w, s=SW):
    """[1024, 1024] f32 -> [128, KT*1024] e4m3 rows, p-major contiguous."""
    q = np.clip(np.asarray(w, np.float32) * s, -240, 240).astype(f8)
    return np.ascontiguousarray(
        q.reshape(KT, 128, 1024).transpose(1, 0, 2).reshape(128, KT * 1024))


def _pack16(w):
    q = np.asarray(w, np.float32).astype(bf16)
    return np.ascontiguousarray(
        q.reshape(KT, 128, 1024).transpose(1, 0, 2).reshape(128, KT * 1024))


def _compose(inp):
    """Fold the two pure-linear layers into their following swiglu weights:
    (x@W0+b0)@W1 + b1 == x@(W0@W1) + (b0@W1 + b1). Exact linear algebra;
    it removes two matmul stages and two quantization round-trips."""
    W0 = np.asarray(inp["lin0_w"], np.float64)
    b0 = np.asarray(inp["lin0_b"], np.float64)
    W1 = np.asarray(inp["lin1_w"], np.float64)
    b1 = np.asarray(inp["lin1_b"], np.float64)
    M1a = W0 @ np.asarray(inp["swi1_w1"], np.float64)
    M1b = W0 @ np.asarray(inp["swi1_w2"], np.float64)
    b1a = b0 @ np.asarray(inp["swi1_w1"], np.float64) + inp["swi1_b1"]
    b1b = b0 @ np.asarray(inp["swi1_w2"], np.float64) + inp["swi1_b2"]
    M2a = W1 @ np.asarray(inp["swi2_w1"], np.float64)
    M2b = W1 @ np.asarray(inp["swi2_w2"], np.float64)
    b2a = b1 @ np.asarray(inp["swi2_w1"], np.float64) + inp["swi2_b1"]
    b2b = b1 @ np.asarray(inp["swi2_w2"], np.float64) + inp["swi2_b2"]
    return (M1a, M1b, M2a, M2b,
            b1a.astype(np.float32), b1b.astype(np.float32),
            b2a.astype(np.float32), b2b.astype(np.float32))


def _packn(w, s):
    """[1024, 1024] -> [128, KT*KT*128] e4m3, n-major then k then c."""
    q = np.clip(np.asarray(w, np.float32) * s, -240, 240).astype(f8)
    q = q.reshape(KT, 128, KT, 128).transpose(1, 2, 0, 3)  # p, n, k, c
    return np.ascontiguousarray(q.reshape(128, KT * KT * 128))


def _pack_weights(inp, M2a, M2b):
    units = []
    w1, w3, w2 = (np.asarray(inp["exp_w1"], np.float32),
                  np.asarray(inp["exp_w3"], np.float32),
                  np.asarray(inp["exp_w2"], np.float32))
    for e in range(E):
        units += [_pack8(w1[e]), _pack8(w3[e]), _pack8(w2[e])]
    sh1 = np.asarray(inp["sh_w1"], np.float32)
    sh3 = np.asarray(inp["sh_w3"], np.float32)
    sh2 = np.asarray(inp["sh_w2"], np.float32)
    units += [_pack8(sh1[:, :1024]), _pack8(sh1[:, 1024:]),
              _pack8(sh3[:, :1024]), _pack8(sh3[:, 1024:]),
              _pack8(sh2[:1024, :]), _pack8(sh2[1024:, :])]
    assert len(units) == N_U8
    wall8 = np.ascontiguousarray(np.concatenate(units, axis=0))
    u16 = [_pack16(M2a), _pack16(M2b)]
    wall16 = np.ascontiguousarray(np.concatenate(u16, axis=0))
    return wall8, wall16


def _pack_biases(b1a, b1b, b2a, b2b):
    cols = []
    for v, s in [(b1a, 1.0), (b1b, SX * SM1), (b2a, 1.0), (b2b, 1.0)]:
        cols.append((v * s).reshape(KT, 128).T.astype(np.float32))
    return np.ascontiguousarray(np.concatenate(cols, axis=1))  # [128, 4*KT]


def kernel(**inputs):
    global _prog
    from concourse.bass_utils import run_bass_kernel_spmd

    if _prog is None:
        _prog = _build_program()
    nc = _prog

    M1a, M1b, M2a, M2b, b1a, b1b, b2a, b2b = _compose(inputs)
    wall8, wall16 = _pack_weights(inputs, M2a, M2b)
    m1an, m1bn = _packn(M1a, SM1), _packn(M1b, SM1)
    biases = _pack_biases(b1a, b1b, b2a, b2b)
    gq = np.clip(np.asarray(inputs["gate_w"], np.float32).T * SW, -240, 240).astype(f8)
    gq = np.ascontiguousarray(
        gq.reshape(KT, 128, E).transpose(1, 0, 2).reshape(128, KT * E))
    ident = np.eye(128, dtype=np.float32)
    sel = np.zeros((E, E * 128), dtype=np.float32)
    for e in range(E):
        sel[e, e * 128:(e + 1) * 128] = 1.0

    x = np.asarray(inputs["x"], np.float32).reshape(T, D)
    in_maps = []
    for c in range(NCORES):
        xT = np.clip(x[c * TPC:(c + 1) * TPC, :].T * SX, -240, 240).astype(f8)
        xqc = np.ascontiguousarray(
            xT.reshape(KT, 128, TPC).transpose(1, 0, 2).reshape(128, KT * TPC))
        in_maps.append({
            "wall8": wall8, "wall16": wall16, "m1an": m1an, "m1bn": m1bn,
            "xq": xqc, "gq": gq, "biases": biases, "ident": ident, "sel": sel,
        })

    res = run_bass_kernel_spmd(nc, in_maps, list(range(NCORES)))
    global last_results
    last_results = res
    outT = np.concatenate([res.results[c]["outT"] for c in range(NCORES)], axis=1)
    return np.ascontiguousarray(outT.T).reshape(B, S, D).astype(np.float32)
